# revision 1
# baseline (speedup 1.0000x reference)
"""Trainium2 Bass kernel: row-parallel linear  y = einsum('sbk,nk->sbn', x, W) + bias.

Strategy
--------
Full inputs arrive on the host. We flatten (seq, batch) -> M = 8192 rows and
shard M across the 8 NeuronCores (1024 rows each); every core streams the full
weight. Each core computes its [1024, 4096] slice of the output with a
split-precision GEMM: x and W are decomposed on the host into bf16 hi/lo pairs
(x = xh + xl exactly to ~16 mantissa bits) and the device accumulates
    y ~= xh*Wh + xh*Wl + xl*Wh
in fp32 PSUM (3 bf16-rate passes; the dropped xl*Wl term is ~2^-18 relative,
i.e. well inside the fp32 envelope for a K=16384 reduction).

Device layout: operands are staged in DRAM as [p=128, ko=K/128, m] with
k = ko*128 + p, so every SBUF tile load is a contiguous-per-partition DMA and
the contraction dim lands on the partition axis, as the PE array requires.

Per core: loop over 4 m-blocks of 256 rows; per m-block the full-K x strip
(hi+lo, 16 MB) stays resident in SBUF while W streams through in
[128, 2048]-column chunks; 8 PSUM banks hold the 2x4 (m-strip x n-tile)
accumulators across the whole K loop, evicted once per block via VectorE.
"""

import os

import numpy as np
import ml_dtypes

BF16 = ml_dtypes.bfloat16

# Problem shapes (hardcoded per contest contract).
SEQ, BATCH, D_FF, D_MODEL = 2048, 4, 16384, 4096
N_CORES = 8
P = 128

M_FULL = SEQ * BATCH            # 8192
M_CORE = M_FULL // N_CORES      # 1024

# Tiling parameters.
M_BLOCK = 256                   # x strip width kept resident in SBUF
N_CHUNK = 2048                  # streamed W chunk width
MM_N = 512                      # matmul free dim (one fp32 PSUM bank)

# Exec-time of the last hardware benchmark (ns), populated when KERNEL_BENCH>0.
LAST_EXEC_NS = None
LAST_RESULTS = None

_BUILD_CACHE = {}
_RUNNER_CACHE = {}


def _build_nc(k, m_core, n, m_block=M_BLOCK, n_chunk=N_CHUNK, reps=1):
    """Build + compile the per-core Bass module for a [m_core, k] x [n, k]^T GEMM.

    reps>1 repeats the whole GEMM inside the program (benchmark variants; the
    timing difference between reps=K and reps=1 isolates steady-state kernel
    time from dispatch overhead)."""
    import concourse.mybir as mybir
    import concourse.tile as tile
    from concourse import bacc

    ko_n = k // P               # number of 128-row k chunks
    n_mb = m_core // m_block
    n_nc = n // n_chunk
    ms_n = m_block // P         # m strips per block
    nt_n = n_chunk // MM_N      # n tiles per chunk
    assert ms_n * nt_n <= 8, "PSUM banks exceeded"

    nc = bacc.Bacc(None, target_bir_lowering=False, debug=False)
    xhi = nc.declare_dram_parameter("xhi", [P, ko_n, m_core], mybir.dt.bfloat16, isOutput=False)
    xlo = nc.declare_dram_parameter("xlo", [P, ko_n, m_core], mybir.dt.bfloat16, isOutput=False)
    whi = nc.declare_dram_parameter("whi", [P, ko_n, n], mybir.dt.bfloat16, isOutput=False)
    wlo = nc.declare_dram_parameter("wlo", [P, ko_n, n], mybir.dt.bfloat16, isOutput=False)
    out = nc.declare_dram_parameter("out", [m_core, n], mybir.dt.float32, isOutput=True)

    f32 = mybir.dt.float32
    bf16 = mybir.dt.bfloat16

    with tile.TileContext(nc) as tc:
        with (
            tc.tile_pool(name="xpool", bufs=1) as xpool,
            tc.tile_pool(name="wpool", bufs=6) as wpool,
            tc.tile_pool(name="opool", bufs=4) as opool,
            tc.tile_pool(name="pspool", bufs=8, space="PSUM") as pspool,
        ):
            for rep, mb in ((r_, m_) for r_ in range(reps) for m_ in range(n_mb)):
                m0 = mb * m_block
                # Resident x strips for this m-block: [P, ko_n, m_block] hi/lo.
                xh = xpool.tile([P, ko_n, m_block], bf16, tag="xh")
                xl = xpool.tile([P, ko_n, m_block], bf16, tag="xl")
                # Load in ko-chunked pieces so the transfer spreads across DMA queues.
                ld_chunk = max(1, ko_n // 8)
                for i in range(0, ko_n, ld_chunk):
                    j = min(i + ld_chunk, ko_n)
                    nc.sync.dma_start(xh[:, i:j, :], xhi[:, i:j, m0:m0 + m_block])
                    nc.sync.dma_start(xl[:, i:j, :], xlo[:, i:j, m0:m0 + m_block])

                for nc0 in range(n_nc):
                    c0 = nc0 * n_chunk
                    psums = [
                        pspool.tile([P, MM_N], f32, tag="ps",
                                    name=f"ps_{rep}_{mb}_{nc0}_{i}")
                        for i in range(ms_n * nt_n)
                    ]
                    for ko in range(ko_n):
                        wh = wpool.tile([P, n_chunk], bf16, tag="wh")
                        wl = wpool.tile([P, n_chunk], bf16, tag="wl")
                        nc.sync.dma_start(wh, whi[:, ko, c0:c0 + n_chunk])
                        nc.sync.dma_start(wl, wlo[:, ko, c0:c0 + n_chunk])
                        first = ko == 0
                        last = ko == ko_n - 1
                        for ms in range(ms_n):
                            lh = xh[:, ko, ms * P:(ms + 1) * P]
                            ll = xl[:, ko, ms * P:(ms + 1) * P]
                            # Rotate PSUM banks on every matmul (consecutive
                            # MMs into the same bank stall the PE) while
                            # keeping the stationary operand grouped.
                            for lhs, w_t, st_flag, sp_flag in (
                                (lh, wh, first, False),
                                (lh, wl, False, False),
                                (ll, wh, False, last),
                            ):
                                for nt in range(nt_n):
                                    nc.tensor.matmul(
                                        psums[ms * nt_n + nt],
                                        lhs,
                                        w_t[:, nt * MM_N:(nt + 1) * MM_N],
                                        start=st_flag,
                                        stop=sp_flag,
                                    )
                    # Evict the 8 accumulators for this (mb, nc0) phase.
                    for ms in range(ms_n):
                        for nt in range(nt_n):
                            st = opool.tile([P, MM_N], f32, tag="st")
                            nc.vector.tensor_copy(out=st, in_=psums[ms * nt_n + nt])
                            nc.sync.dma_start(
                                out[m0 + ms * P:m0 + (ms + 1) * P,
                                    c0 + nt * MM_N:c0 + (nt + 1) * MM_N],
                                st,
                            )
    nc.compile()
    return nc


def _build_nc_v2(k, m_core, n, kb_n=4, reps=1):
    """K-blocked variant: full-m x block resident per K-block, W streamed
    exactly once, output accumulated across K-blocks in DRAM via SWDGE
    CCE-add. Total HBM traffic ~2.5x lower than _build_nc."""
    import concourse.mybir as mybir
    import concourse.tile as tile
    from concourse import bacc

    ko_n = k // P            # 128-row k chunks overall
    ko_b = ko_n // kb_n      # k chunks per block
    ms_n = m_core // P       # m strips (psum tiles per chunk)
    nb_n = n // MM_N         # 512-wide n chunks
    assert ms_n <= 8, "PSUM banks exceeded"

    nc = bacc.Bacc(None, target_bir_lowering=False, debug=False)
    xhi = nc.declare_dram_parameter("xhi", [P, ko_n, m_core], mybir.dt.bfloat16, isOutput=False)
    xlo = nc.declare_dram_parameter("xlo", [P, ko_n, m_core], mybir.dt.bfloat16, isOutput=False)
    whi = nc.declare_dram_parameter("whi", [P, ko_n, n], mybir.dt.bfloat16, isOutput=False)
    wlo = nc.declare_dram_parameter("wlo", [P, ko_n, n], mybir.dt.bfloat16, isOutput=False)
    out = nc.declare_dram_parameter("out", [m_core, n], mybir.dt.float32, isOutput=True)

    f32 = mybir.dt.float32
    bf16 = mybir.dt.bfloat16
    add = mybir.AluOpType.add

    with tile.TileContext(nc) as tc:
        with (
            tc.tile_pool(name="xpool", bufs=2 * ko_b + 4) as xpool,
            tc.tile_pool(name="wpool", bufs=8) as wpool,
            tc.tile_pool(name="opool", bufs=4) as opool,
            tc.tile_pool(name="pspool", bufs=8, space="PSUM") as pspool,
        ):
            for rep in range(reps):
                for kb in range(kb_n):
                    k0 = kb * ko_b
                    # Resident x tiles for this K-block: one [P, m_core] tile
                    # per (ko, hi/lo). Spare pool slots let the next block's
                    # first chunks prefetch under this block's tail.
                    xts = []
                    for i in range(ko_b):
                        xh = xpool.tile([P, m_core], bf16, tag="xt",
                                        name=f"xh_{rep}_{kb}_{i}")
                        xl = xpool.tile([P, m_core], bf16, tag="xt",
                                        name=f"xl_{rep}_{kb}_{i}")
                        nc.sync.dma_start(xh, xhi[:, k0 + i, :])
                        nc.sync.dma_start(xl, xlo[:, k0 + i, :])
                        xts.append((xh, xl))
                    for nb in range(nb_n):
                        c0 = nb * MM_N
                        psums = [
                            pspool.tile([P, MM_N], f32, tag="ps",
                                        name=f"ps_{rep}_{kb}_{nb}_{i}")
                            for i in range(ms_n)
                        ]
                        for i in range(ko_b):
                            wh = wpool.tile([P, MM_N], bf16, tag="wh")
                            wl = wpool.tile([P, MM_N], bf16, tag="wl")
                            nc.sync.dma_start(wh, whi[:, k0 + i, c0:c0 + MM_N])
                            nc.sync.dma_start(wl, wlo[:, k0 + i, c0:c0 + MM_N])
                            first = i == 0
                            last = i == ko_b - 1
                            xh, xl = xts[i]
                            # Bank-rotating order: consecutive MMs never hit
                            # the same PSUM bank.
                            for hi_lo, w_t, st_flag, sp_flag in (
                                (0, wh, first, False),
                                (0, wl, False, False),
                                (1, wh, False, last),
                            ):
                                for ms in range(ms_n):
                                    src = xts[i][hi_lo]
                                    nc.tensor.matmul(
                                        psums[ms],
                                        src[:, ms * P:(ms + 1) * P],
                                        w_t,
                                        start=st_flag,
                                        stop=sp_flag,
                                    )
                        for ms in range(ms_n):
                            st = opool.tile([P, MM_N], f32, tag="st")
                            nc.vector.tensor_copy(out=st, in_=psums[ms])
                            dst = out[ms * P:(ms + 1) * P, c0:c0 + MM_N]
                            if kb == 0 and rep == 0:
                                nc.sync.dma_start(dst, st)
                            else:
                                nc.gpsimd.dma_start(dst, st, accum_op=add)
    nc.compile()
    return nc


def _get_nc(k, m_core, n, **kw):
    variant = os.environ.get("KERNEL_VARIANT", "v1")
    key = (variant, k, m_core, n, tuple(sorted(kw.items())))
    if key not in _BUILD_CACHE:
        build = _build_nc_v2 if variant == "v2" else _build_nc
        _BUILD_CACHE[key] = build(k, m_core, n, **kw)
    return _BUILD_CACHE[key]


def _split_bf16(a_f32):
    """Exact split a = hi + lo with hi, lo bf16 (a contiguous fp32 array)."""
    hi = a_f32.astype(BF16)
    lo = (a_f32 - hi.astype(np.float32)).astype(BF16)
    return hi, lo


def _to_pkm(a, ko_n):
    """[rows, k] fp32 -> contiguous [P, ko_n, rows] (k = ko*128 + p)."""
    rows = a.shape[0]
    return np.ascontiguousarray(a.reshape(rows, ko_n, P).transpose(2, 1, 0))


def _make_runner(nc):
    """Build the sharded PJRT executor for `nc` across the 8 cores.

    Mirrors concourse.bass2jax.run_bass_via_pjrt, but returns a reusable
    closure so repeated calls share one jit cache and inputs can stay
    device-resident for benchmarking.
    """
    import jax
    import concourse.mybir as mybir
    from concourse import bass2jax
    from jax.experimental.shard_map import shard_map
    from jax.sharding import Mesh, NamedSharding, PartitionSpec

    bass2jax.install_neuronx_cc_hook()

    partition_name = nc.partition_id_tensor.name if nc.partition_id_tensor else None
    assert nc.dbg_addr is None

    in_names, out_names, out_avals = [], [], []
    for alloc in nc.m.functions[0].allocations:
        if not isinstance(alloc, mybir.MemoryLocationSet):
            continue
        name = alloc.memorylocations[0].name
        if alloc.kind == "ExternalInput":
            if name != partition_name:
                in_names.append(name)
        elif alloc.kind == "ExternalOutput":
            out_names.append(name)
            out_avals.append(
                jax.core.ShapedArray(tuple(alloc.tensor_shape), mybir.dt.np(alloc.dtype))
            )
    n_params = len(in_names)
    n_outs = len(out_avals)
    all_in_names = tuple(in_names) + tuple(out_names)
    if partition_name is not None:
        all_in_names = all_in_names + (partition_name,)
    donate = tuple(range(n_params, n_params + n_outs))

    def _body(*args):
        operands = list(args)
        if partition_name is not None:
            operands.append(bass2jax.partition_id_tensor())
        outs = bass2jax._bass_exec_p.bind(
            *operands,
            out_avals=tuple(out_avals),
            in_names=all_in_names,
            out_names=tuple(out_names),
            lowering_input_output_aliases=(),
            sim_require_finite=True,
            sim_require_nnan=True,
            nc=nc,
        )
        return tuple(outs)

    devices = jax.devices()[:N_CORES]
    assert len(devices) == N_CORES
    mesh = Mesh(np.asarray(devices), ("core",))
    spec = PartitionSpec("core")
    sharded = jax.jit(
        shard_map(
            _body,
            mesh=mesh,
            in_specs=(spec,) * (n_params + n_outs),
            out_specs=(spec,) * n_outs,
            check_rep=False,
        ),
        donate_argnums=donate,
        keep_unused=True,
    )
    sharding = NamedSharding(mesh, spec)
    return {
        "sharded": sharded,
        "sharding": sharding,
        "in_names": in_names,
        "out_names": out_names,
        "out_avals": out_avals,
        "n_params": n_params,
        "n_outs": n_outs,
    }


def _get_runner(nc):
    key = id(nc)
    if key not in _RUNNER_CACHE:
        _RUNNER_CACHE[key] = _make_runner(nc)
    return _RUNNER_CACHE[key]


def _run(nc, in_maps):
    """Execute the kernel across 8 cores; returns per-core output dicts."""
    import numpy as np

    r = _get_runner(nc)
    n_cores = len(in_maps)
    concat_in = [
        np.concatenate([np.asarray(m[name]) for m in in_maps], axis=0)
        for name in r["in_names"]
    ]
    concat_zeros = [
        np.zeros((n_cores * a.shape[0], *a.shape[1:]), a.dtype) for a in r["out_avals"]
    ]
    out_arrs = r["sharded"](*concat_in, *concat_zeros)
    return [
        {
            name: np.asarray(out_arrs[i]).reshape(n_cores, *r["out_avals"][i].shape)[c]
            for i, name in enumerate(r["out_names"])
        }
        for c in range(n_cores)
    ]


def _bench(in_maps, k, m_core, n, reps):
    """Measure steady-state per-GEMM time: the kernel repeated `reps` times
    inside one program, minus the reps=1 program, divided by reps-1. Fixed
    dispatch overhead cancels in the difference. Sets LAST_EXEC_NS."""
    global LAST_EXEC_NS
    import time

    import jax
    import jax.numpy as jnp
    import numpy as np

    times = {}
    dev_in = None
    for r_reps in (1, reps):
        nc = _get_nc(k, m_core, n, reps=r_reps)
        r = _get_runner(nc)
        if dev_in is None:
            concat_in = [
                np.concatenate([np.asarray(m[name]) for m in in_maps], axis=0)
                for name in r["in_names"]
            ]
            dev_in = [jax.device_put(a, r["sharding"]) for a in concat_in]
            jax.block_until_ready(dev_in)

        def _zeros():
            zs = [
                jax.jit(lambda a=a: jnp.zeros(a.shape, a.dtype),
                        out_shardings=r["sharding"])()
                for a in r["out_avals"]
            ]
            jax.block_until_ready(zs)
            return zs

        out = r["sharded"](*dev_in, *_zeros())  # compile + warmup
        jax.block_until_ready(out)
        best = float("inf")
        for _ in range(3):
            zs = _zeros()
            t0 = time.perf_counter()
            out = r["sharded"](*dev_in, *zs)
            jax.block_until_ready(out)
            best = min(best, time.perf_counter() - t0)
        times[r_reps] = best
        print(f"[bench] reps={r_reps}: {best * 1e3:.3f} ms")

    per_iter = (times[reps] - times[1]) / (reps - 1)
    LAST_EXEC_NS = int(per_iter * 1e9)
    print(f"[bench] per-GEMM: {per_iter * 1e3:.3f} ms "
          f"(fixed+1iter: {times[1] * 1e3:.3f} ms)")


def kernel(input_, weight, bias):
    global LAST_RESULTS

    input_ = np.asarray(input_, dtype=np.float32)
    weight = np.asarray(weight, dtype=np.float32)
    bias = np.asarray(bias, dtype=np.float32)

    seq, batch, k = input_.shape
    n = weight.shape[0]
    m_full = seq * batch
    m_core = m_full // N_CORES
    ko_n = k // P

    nc = _get_nc(k, m_core, n)

    x2 = input_.reshape(m_full, k)
    wT = _to_pkm(weight, ko_n)                  # [P, ko, n] fp32
    whi, wlo = _split_bf16(wT)
    del wT

    in_maps = []
    for c in range(N_CORES):
        xcT = _to_pkm(x2[c * m_core:(c + 1) * m_core], ko_n)  # [P, ko, m_core]
        xh, xl = _split_bf16(xcT)
        del xcT
        in_maps.append({"xhi": xh, "xlo": xl, "whi": whi, "wlo": wlo})

    results = _run(nc, in_maps)
    LAST_RESULTS = results

    bench_reps = int(os.environ.get("KERNEL_BENCH", "0"))
    if bench_reps > 1:
        _bench(in_maps, k, m_core, n, bench_reps)

    out = np.concatenate([results[c]["out"] for c in range(N_CORES)], axis=0)
    out = out.reshape(seq, batch, n)
    if bias.any():
        out = out + bias
    return out



# revision 2
# speedup vs baseline: 1.9365x; 1.9365x over previous
"""Trainium2 Bass kernel: row-parallel linear  y = einsum('sbk,nk->sbn', x, W) + bias.

Strategy
--------
Full inputs arrive on the host. We flatten (seq, batch) -> M = 8192 rows and
shard M across the 8 NeuronCores (1024 rows each); every core streams the full
weight and computes its [1024, 4096] slice of the output.

The correctness gate is rel_err < 2e-2 (max-abs over max-abs), which a single
reduced-precision GEMM pass meets comfortably:
  - "bf1": one bf16 pass              (measured ~9e-4 rel err)
  - "fp8": one e4m3 DoubleRow pass    (measured ~1.5e-2 rel err) at ~1.5-1.8x
    the bf16 matmul rate (256-deep contraction per PE instruction).
Host-side quantization makes the device error deterministic: products are
exact in fp8/bf16 and accumulate in fp32 PSUM.

Device layout: operands are staged in DRAM with the contraction dim on the
partition axis: x as [n_mb, P, ko, m_block] and W as [P, ko, n] with
k = ko*128 + p, so every SBUF tile load is contiguous-per-partition.

Per core: loop over m-blocks; per m-block the full-K x strip stays resident
in SBUF (loaded as ko-chunked tiles so matmuls start as soon as their chunk
lands and the next block prefetches into spare pool slots); W streams through
once per m-block; 8 PSUM banks hold the (m-strip x n-tile) accumulators
across the whole K loop, evicted once per n-chunk via VectorE.
"""

import os

import numpy as np
import ml_dtypes

BF16 = ml_dtypes.bfloat16
E4M3 = ml_dtypes.float8_e4m3  # TRN semantics: max normal +-240

# Problem shapes (hardcoded per contest contract).
SEQ, BATCH, D_FF, D_MODEL = 2048, 4, 16384, 4096
N_CORES = 8
P = 128

M_FULL = SEQ * BATCH            # 8192
M_CORE = M_FULL // N_CORES      # 1024

MM_N = 512                      # matmul free dim (one fp32 PSUM bank)
KO_LD = 16                      # ko chunks per x load tile

W_SCALE = 128.0                 # fp8: weight pre-scale (power of two, exact)

# Exec-time of the last hardware benchmark (ns), populated when KERNEL_BENCH>0.
LAST_EXEC_NS = None
LAST_RESULTS = None

_BUILD_CACHE = {}
_RUNNER_CACHE = {}


def _build_bf1(k, m_core, n, m_block=512, n_chunk=1024, reps=1):
    """Single-pass bf16 GEMM: out[m_core, n] = x[m_core, k] @ w[n, k]^T.

    PSUM holds (m_block/128) x (n_chunk/512) fp32 accumulators across the
    full K loop; consecutive matmuls rotate banks. W is streamed once per
    m-block; x tiles are ko-chunked for fine-grained deps + prefetch."""
    import concourse.mybir as mybir
    import concourse.tile as tile
    from concourse import bacc

    ko_n = k // P
    n_mb = m_core // m_block
    n_nc = n // n_chunk
    ms_n = m_block // P
    nt_n = n_chunk // MM_N
    n_ld = ko_n // KO_LD
    assert ms_n * nt_n <= 8, "PSUM banks exceeded"

    nc = bacc.Bacc(None, target_bir_lowering=False, debug=False)
    xb = nc.declare_dram_parameter("xb", [n_mb, P, ko_n, m_block],
                                   mybir.dt.bfloat16, isOutput=False)
    wb = nc.declare_dram_parameter("wb", [P, ko_n, n],
                                   mybir.dt.bfloat16, isOutput=False)
    out = nc.declare_dram_parameter("out", [m_core, n], mybir.dt.float32,
                                    isOutput=True)

    f32 = mybir.dt.float32
    bf16 = mybir.dt.bfloat16

    with tile.TileContext(nc) as tc:
        with (
            tc.tile_pool(name="xpool", bufs=n_ld + 2) as xpool,
            tc.tile_pool(name="wpool", bufs=6) as wpool,
            tc.tile_pool(name="opool", bufs=4) as opool,
            tc.tile_pool(name="pspool", bufs=8, space="PSUM") as pspool,
        ):
            for rep, mb in ((r_, m_) for r_ in range(reps) for m_ in range(n_mb)):
                xts = []
                for i in range(n_ld):
                    xt = xpool.tile([P, KO_LD, m_block], bf16, tag="xt",
                                    name=f"x_{rep}_{mb}_{i}")
                    nc.sync.dma_start(xt, xb[mb, :, i * KO_LD:(i + 1) * KO_LD, :])
                    xts.append(xt)
                m0 = mb * m_block
                for nc0 in range(n_nc):
                    c0 = nc0 * n_chunk
                    psums = [
                        pspool.tile([P, MM_N], f32, tag="ps",
                                    name=f"ps_{rep}_{mb}_{nc0}_{i}")
                        for i in range(ms_n * nt_n)
                    ]
                    for ko in range(ko_n):
                        wt = wpool.tile([P, n_chunk], bf16, tag="wt")
                        nc.sync.dma_start(wt, wb[:, ko, c0:c0 + n_chunk])
                        first = ko == 0
                        last = ko == ko_n - 1
                        xt = xts[ko // KO_LD]
                        kj = ko % KO_LD
                        for ms in range(ms_n):
                            lhs = xt[:, kj, ms * P:(ms + 1) * P]
                            for nt in range(nt_n):
                                nc.tensor.matmul(
                                    psums[ms * nt_n + nt],
                                    lhs,
                                    wt[:, nt * MM_N:(nt + 1) * MM_N],
                                    start=first,
                                    stop=last,
                                )
                    for ms in range(ms_n):
                        for nt in range(nt_n):
                            st = opool.tile([P, MM_N], f32, tag="st")
                            nc.vector.tensor_copy(out=st, in_=psums[ms * nt_n + nt])
                            nc.sync.dma_start(
                                out[m0 + ms * P:m0 + (ms + 1) * P,
                                    c0 + nt * MM_N:c0 + (nt + 1) * MM_N],
                                st,
                            )
    nc.compile()
    return nc


def _build_fp8(k, m_core, n, m_block=256, n_chunk=2048, reps=1):
    """Single-pass e4m3 GEMM with DoubleRow: each matmul contracts 256 rows
    (2 ko chunks packed per PE cell). Both operands carry a [P, 2, free] AP.
    W arrives pre-scaled by W_SCALE; the host descales the output."""
    import concourse.mybir as mybir
    import concourse.tile as tile
    from concourse import bacc

    ko_n = k // P
    kp_n = ko_n // 2            # ko pairs
    n_mb = m_core // m_block
    n_nc = n // n_chunk
    ms_n = m_block // P
    nt_n = n_chunk // MM_N
    n_ld = ko_n // KO_LD
    assert ms_n * nt_n <= 8, "PSUM banks exceeded"

    nc = bacc.Bacc(None, target_bir_lowering=False, debug=False)
    xb = nc.declare_dram_parameter("xb", [n_mb, P, ko_n, m_block],
                                   mybir.dt.float8e4, isOutput=False)
    wb = nc.declare_dram_parameter("wb", [P, ko_n, n],
                                   mybir.dt.float8e4, isOutput=False)
    out = nc.declare_dram_parameter("out", [m_core, n], mybir.dt.float32,
                                    isOutput=True)

    f32 = mybir.dt.float32
    fp8 = mybir.dt.float8e4
    dr = mybir.MatmulPerfMode.DoubleRow

    with tile.TileContext(nc) as tc:
        with (
            tc.tile_pool(name="xpool", bufs=n_ld + 2) as xpool,
            tc.tile_pool(name="wpool", bufs=6) as wpool,
            tc.tile_pool(name="opool", bufs=4) as opool,
            tc.tile_pool(name="pspool", bufs=8, space="PSUM") as pspool,
        ):
            for rep, mb in ((r_, m_) for r_ in range(reps) for m_ in range(n_mb)):
                xts = []
                for i in range(n_ld):
                    xt = xpool.tile([P, KO_LD, m_block], fp8, tag="xt",
                                    name=f"x_{rep}_{mb}_{i}")
                    nc.sync.dma_start(xt, xb[mb, :, i * KO_LD:(i + 1) * KO_LD, :])
                    xts.append(xt)
                m0 = mb * m_block
                for nc0 in range(n_nc):
                    c0 = nc0 * n_chunk
                    psums = [
                        pspool.tile([P, MM_N], f32, tag="ps",
                                    name=f"ps_{rep}_{mb}_{nc0}_{i}")
                        for i in range(ms_n * nt_n)
                    ]
                    for kp in range(kp_n):
                        wt = wpool.tile([P, 2, n_chunk], fp8, tag="wt")
                        nc.sync.dma_start(wt, wb[:, 2 * kp:2 * kp + 2,
                                                 c0:c0 + n_chunk])
                        first = kp == 0
                        last = kp == kp_n - 1
                        xt = xts[(2 * kp) // KO_LD]
                        kj = (2 * kp) % KO_LD
                        for ms in range(ms_n):
                            lhs = xt[:, kj:kj + 2, ms * P:(ms + 1) * P]
                            for nt in range(nt_n):
                                nc.tensor.matmul(
                                    psums[ms * nt_n + nt],
                                    lhs,
                                    wt[:, :, nt * MM_N:(nt + 1) * MM_N],
                                    start=first,
                                    stop=last,
                                    perf_mode=dr,
                                )
                    for ms in range(ms_n):
                        for nt in range(nt_n):
                            st = opool.tile([P, MM_N], f32, tag="st")
                            nc.vector.tensor_copy(out=st, in_=psums[ms * nt_n + nt])
                            nc.sync.dma_start(
                                out[m0 + ms * P:m0 + (ms + 1) * P,
                                    c0 + nt * MM_N:c0 + (nt + 1) * MM_N],
                                st,
                            )
    nc.compile()
    return nc


_BUILDERS = {"bf1": _build_bf1, "fp8": _build_fp8}


def _variant():
    return os.environ.get("KERNEL_VARIANT", "bf1")


def _get_nc(k, m_core, n, **kw):
    variant = _variant()
    key = (variant, k, m_core, n, tuple(sorted(kw.items())))
    if key not in _BUILD_CACHE:
        _BUILD_CACHE[key] = _BUILDERS[variant](k, m_core, n, **kw)
    return _BUILD_CACHE[key]


def _to_pkm_blocks(a, m_block, dtype):
    """[rows, k] fp32 -> contiguous [n_mb, P, ko_n, m_block] in `dtype`
    (k = ko*128 + p)."""
    rows, k = a.shape
    n_mb = rows // m_block
    ko_n = k // P
    a = a.astype(dtype)
    a = a.reshape(n_mb, m_block, ko_n, P).transpose(0, 3, 2, 1)
    return np.ascontiguousarray(a)


def _w_to_pkn(w, dtype, scale=1.0):
    """[n, k] fp32 -> contiguous [P, ko_n, n] in `dtype`."""
    n, k = w.shape
    ko_n = k // P
    if scale != 1.0:
        w = w * np.float32(scale)
    w = w.astype(dtype)
    w = w.reshape(n, ko_n, P).transpose(2, 1, 0)
    return np.ascontiguousarray(w)


def _make_runner(nc):
    """Build the sharded PJRT executor for `nc` across the 8 cores.

    Mirrors concourse.bass2jax.run_bass_via_pjrt, but returns a reusable
    closure so repeated calls share one jit cache and inputs can stay
    device-resident for benchmarking.
    """
    import jax
    import concourse.mybir as mybir
    from concourse import bass2jax
    from jax.experimental.shard_map import shard_map
    from jax.sharding import Mesh, NamedSharding, PartitionSpec

    bass2jax.install_neuronx_cc_hook()

    partition_name = nc.partition_id_tensor.name if nc.partition_id_tensor else None
    assert nc.dbg_addr is None

    in_names, out_names, out_avals = [], [], []
    for alloc in nc.m.functions[0].allocations:
        if not isinstance(alloc, mybir.MemoryLocationSet):
            continue
        name = alloc.memorylocations[0].name
        if alloc.kind == "ExternalInput":
            if name != partition_name:
                in_names.append(name)
        elif alloc.kind == "ExternalOutput":
            out_names.append(name)
            out_avals.append(
                jax.core.ShapedArray(tuple(alloc.tensor_shape), mybir.dt.np(alloc.dtype))
            )
    n_params = len(in_names)
    n_outs = len(out_avals)
    all_in_names = tuple(in_names) + tuple(out_names)
    if partition_name is not None:
        all_in_names = all_in_names + (partition_name,)
    donate = tuple(range(n_params, n_params + n_outs))

    def _body(*args):
        operands = list(args)
        if partition_name is not None:
            operands.append(bass2jax.partition_id_tensor())
        outs = bass2jax._bass_exec_p.bind(
            *operands,
            out_avals=tuple(out_avals),
            in_names=all_in_names,
            out_names=tuple(out_names),
            lowering_input_output_aliases=(),
            sim_require_finite=True,
            sim_require_nnan=True,
            nc=nc,
        )
        return tuple(outs)

    devices = jax.devices()[:N_CORES]
    assert len(devices) == N_CORES
    mesh = Mesh(np.asarray(devices), ("core",))
    spec = PartitionSpec("core")
    sharded = jax.jit(
        shard_map(
            _body,
            mesh=mesh,
            in_specs=(spec,) * (n_params + n_outs),
            out_specs=(spec,) * n_outs,
            check_rep=False,
        ),
        donate_argnums=donate,
        keep_unused=True,
    )
    sharding = NamedSharding(mesh, spec)
    return {
        "sharded": sharded,
        "sharding": sharding,
        "in_names": in_names,
        "out_names": out_names,
        "out_avals": out_avals,
        "n_params": n_params,
        "n_outs": n_outs,
    }


def _get_runner(nc):
    key = id(nc)
    if key not in _RUNNER_CACHE:
        _RUNNER_CACHE[key] = _make_runner(nc)
    return _RUNNER_CACHE[key]


def _run(nc, in_maps):
    """Execute the kernel across 8 cores; returns per-core output dicts."""
    import numpy as np

    r = _get_runner(nc)
    n_cores = len(in_maps)
    concat_in = [
        np.concatenate([np.asarray(m[name]) for m in in_maps], axis=0)
        for name in r["in_names"]
    ]
    concat_zeros = [
        np.zeros((n_cores * a.shape[0], *a.shape[1:]), a.dtype) for a in r["out_avals"]
    ]
    out_arrs = r["sharded"](*concat_in, *concat_zeros)
    return [
        {
            name: np.asarray(out_arrs[i]).reshape(n_cores, *r["out_avals"][i].shape)[c]
            for i, name in enumerate(r["out_names"])
        }
        for c in range(n_cores)
    ]


def _bench(in_maps, k, m_core, n, reps):
    """Measure steady-state per-GEMM time: the kernel repeated `reps` times
    inside one program, minus the reps=1 program, divided by reps-1. Fixed
    dispatch overhead cancels in the difference. Sets LAST_EXEC_NS."""
    global LAST_EXEC_NS
    import time

    import jax
    import jax.numpy as jnp
    import numpy as np

    times = {}
    dev_in = None
    for r_reps in (1, reps):
        nc = _get_nc(k, m_core, n, reps=r_reps)
        r = _get_runner(nc)
        if dev_in is None:
            concat_in = [
                np.concatenate([np.asarray(m[name]) for m in in_maps], axis=0)
                for name in r["in_names"]
            ]
            dev_in = [jax.device_put(a, r["sharding"]) for a in concat_in]
            jax.block_until_ready(dev_in)

        def _zeros():
            zs = [
                jax.jit(lambda a=a: jnp.zeros(a.shape, a.dtype),
                        out_shardings=r["sharding"])()
                for a in r["out_avals"]
            ]
            jax.block_until_ready(zs)
            return zs

        out = r["sharded"](*dev_in, *_zeros())  # compile + warmup
        jax.block_until_ready(out)
        attempts = []
        for _ in range(int(os.environ.get("KERNEL_BENCH_TRIES", "5"))):
            zs = _zeros()
            t0 = time.perf_counter()
            out = r["sharded"](*dev_in, *zs)
            jax.block_until_ready(out)
            attempts.append(time.perf_counter() - t0)
        times[r_reps] = min(attempts)
        print(f"[bench] reps={r_reps}: best {min(attempts) * 1e3:.3f} ms  "
              f"all {[f'{a * 1e3:.2f}' for a in attempts]}")

    per_iter = (times[reps] - times[1]) / (reps - 1)
    LAST_EXEC_NS = int(per_iter * 1e9)
    print(f"[bench] per-GEMM: {per_iter * 1e3:.3f} ms "
          f"(fixed+1iter: {times[1] * 1e3:.3f} ms)")


def kernel(input_, weight, bias):
    global LAST_RESULTS

    input_ = np.asarray(input_, dtype=np.float32)
    weight = np.asarray(weight, dtype=np.float32)
    bias = np.asarray(bias, dtype=np.float32)

    seq, batch, k = input_.shape
    n = weight.shape[0]
    m_full = seq * batch
    m_core = m_full // N_CORES

    variant = _variant()
    nc = _get_nc(k, m_core, n)

    x2 = input_.reshape(m_full, k)
    if variant == "fp8":
        m_block, dtype, w_scale = 256, E4M3, W_SCALE
    else:
        m_block, dtype, w_scale = 512, BF16, 1.0
    wp = _w_to_pkn(weight, dtype, scale=w_scale)

    in_maps = []
    for c in range(N_CORES):
        xp = _to_pkm_blocks(x2[c * m_core:(c + 1) * m_core], m_block, dtype)
        in_maps.append({"xb": xp, "wb": wp})

    results = _run(nc, in_maps)
    LAST_RESULTS = results

    bench_reps = int(os.environ.get("KERNEL_BENCH", "0"))
    if bench_reps > 1:
        _bench(in_maps, k, m_core, n, bench_reps)

    out = np.concatenate([results[c]["out"] for c in range(N_CORES)], axis=0)
    if w_scale != 1.0:
        out = out * np.float32(1.0 / w_scale)
    out = out.reshape(seq, batch, n)
    if bias.any():
        out = out + bias
    return out


# revision 3
# speedup vs baseline: 2.9957x; 1.5470x over previous
"""Trainium2 Bass kernel: row-parallel linear  y = einsum('sbk,nk->sbn', x, W) + bias.

Strategy
--------
Full inputs arrive on the host. We flatten (seq, batch) -> M = 8192 rows and
shard M across the 8 NeuronCores (1024 rows each); every core streams the full
weight and computes its [1024, 4096] slice of the output.

The correctness gate is rel_err < 2e-2 (max-abs over max-abs), which a single
reduced-precision GEMM pass meets comfortably:
  - "bf1": one bf16 pass              (measured ~9e-4 rel err)
  - "fp8": one e4m3 DoubleRow pass    (measured ~1.5e-2 rel err) at ~1.5-1.8x
    the bf16 matmul rate (256-deep contraction per PE instruction).
Host-side quantization makes the device error deterministic: products are
exact in fp8/bf16 and accumulate in fp32 PSUM.

Device layout: operands are staged in DRAM with the contraction dim on the
partition axis: x as [n_mb, P, ko, m_block] and W as [P, ko, n] with
k = ko*128 + p, so every SBUF tile load is contiguous-per-partition.

Per core: loop over m-blocks; per m-block the full-K x strip stays resident
in SBUF (loaded as ko-chunked tiles so matmuls start as soon as their chunk
lands and the next block prefetches into spare pool slots); W streams through
once per m-block; 8 PSUM banks hold the (m-strip x n-tile) accumulators
across the whole K loop, evicted once per n-chunk via VectorE.
"""

import os

import numpy as np
import ml_dtypes

BF16 = ml_dtypes.bfloat16
E4M3 = ml_dtypes.float8_e4m3  # TRN semantics: max normal +-240

# Problem shapes (hardcoded per contest contract).
SEQ, BATCH, D_FF, D_MODEL = 2048, 4, 16384, 4096
N_CORES = 8
P = 128

M_FULL = SEQ * BATCH            # 8192
M_CORE = M_FULL // N_CORES      # 1024

MM_N = 512                      # matmul free dim (one fp32 PSUM bank)
KO_LD = 16                      # ko chunks per x load tile

W_SCALE = 128.0                 # fp8: weight pre-scale (power of two, exact)

# Exec-time of the last hardware benchmark (ns), populated when KERNEL_BENCH>0.
LAST_EXEC_NS = None
LAST_RESULTS = None

_BUILD_CACHE = {}
_RUNNER_CACHE = {}


def _build_bf1(k, m_core, n, m_block=512, n_chunk=1024, reps=1):
    """Single-pass bf16 GEMM: out[m_core, n] = x[m_core, k] @ w[n, k]^T.

    PSUM holds (m_block/128) x (n_chunk/512) fp32 accumulators across the
    full K loop; consecutive matmuls rotate banks. W is streamed once per
    m-block; x tiles are ko-chunked for fine-grained deps + prefetch."""
    import concourse.mybir as mybir
    import concourse.tile as tile
    from concourse import bacc

    ko_n = k // P
    n_mb = m_core // m_block
    n_nc = n // n_chunk
    ms_n = m_block // P
    nt_n = n_chunk // MM_N
    n_ld = ko_n // KO_LD
    assert ms_n * nt_n <= 8, "PSUM banks exceeded"

    nc = bacc.Bacc(None, target_bir_lowering=False, debug=False)
    xb = nc.declare_dram_parameter("xb", [n_mb, P, ko_n, m_block],
                                   mybir.dt.bfloat16, isOutput=False)
    wb = nc.declare_dram_parameter("wb", [P, ko_n, n],
                                   mybir.dt.bfloat16, isOutput=False)
    out = nc.declare_dram_parameter("out", [m_core, n], mybir.dt.float32,
                                    isOutput=True)

    f32 = mybir.dt.float32
    bf16 = mybir.dt.bfloat16

    with tile.TileContext(nc) as tc:
        with (
            tc.tile_pool(name="xpool", bufs=n_ld + 2) as xpool,
            tc.tile_pool(name="wpool", bufs=6) as wpool,
            tc.tile_pool(name="opool", bufs=4) as opool,
            tc.tile_pool(name="pspool", bufs=8, space="PSUM") as pspool,
        ):
            for rep, mb in ((r_, m_) for r_ in range(reps) for m_ in range(n_mb)):
                xts = []
                for i in range(n_ld):
                    xt = xpool.tile([P, KO_LD, m_block], bf16, tag="xt",
                                    name=f"x_{rep}_{mb}_{i}")
                    nc.sync.dma_start(xt, xb[mb, :, i * KO_LD:(i + 1) * KO_LD, :])
                    xts.append(xt)
                m0 = mb * m_block
                for nc0 in range(n_nc):
                    c0 = nc0 * n_chunk
                    psums = [
                        pspool.tile([P, MM_N], f32, tag="ps",
                                    name=f"ps_{rep}_{mb}_{nc0}_{i}")
                        for i in range(ms_n * nt_n)
                    ]
                    for ko in range(ko_n):
                        wt = wpool.tile([P, n_chunk], bf16, tag="wt")
                        nc.sync.dma_start(wt, wb[:, ko, c0:c0 + n_chunk])
                        first = ko == 0
                        last = ko == ko_n - 1
                        xt = xts[ko // KO_LD]
                        kj = ko % KO_LD
                        for ms in range(ms_n):
                            lhs = xt[:, kj, ms * P:(ms + 1) * P]
                            for nt in range(nt_n):
                                nc.tensor.matmul(
                                    psums[ms * nt_n + nt],
                                    lhs,
                                    wt[:, nt * MM_N:(nt + 1) * MM_N],
                                    start=first,
                                    stop=last,
                                )
                    for ms in range(ms_n):
                        for nt in range(nt_n):
                            st = opool.tile([P, MM_N], f32, tag="st")
                            nc.vector.tensor_copy(out=st, in_=psums[ms * nt_n + nt])
                            nc.sync.dma_start(
                                out[m0 + ms * P:m0 + (ms + 1) * P,
                                    c0 + nt * MM_N:c0 + (nt + 1) * MM_N],
                                st,
                            )
    nc.compile()
    return nc


def _build_fp8(k, m_core, n, m_block=256, n_chunk=2048, reps=1):
    """Single-pass e4m3 GEMM with DoubleRow: each matmul contracts 256 rows
    (2 ko chunks packed per PE cell). Both operands carry a [P, 2, free] AP.
    W arrives pre-scaled by W_SCALE; the host descales the output."""
    import concourse.mybir as mybir
    import concourse.tile as tile
    from concourse import bacc

    ko_n = k // P
    kp_n = ko_n // 2            # ko pairs
    n_mb = m_core // m_block
    n_nc = n // n_chunk
    ms_n = m_block // P
    nt_n = n_chunk // MM_N
    n_ld = ko_n // KO_LD
    assert ms_n * nt_n <= 8, "PSUM banks exceeded"

    nc = bacc.Bacc(None, target_bir_lowering=False, debug=False)
    xb = nc.declare_dram_parameter("xb", [n_mb, P, ko_n, m_block],
                                   mybir.dt.float8e4, isOutput=False)
    wb = nc.declare_dram_parameter("wb", [P, ko_n, n],
                                   mybir.dt.float8e4, isOutput=False)
    out = nc.declare_dram_parameter("out", [m_core, n], mybir.dt.float32,
                                    isOutput=True)

    f32 = mybir.dt.float32
    fp8 = mybir.dt.float8e4
    dr = mybir.MatmulPerfMode.DoubleRow

    with tile.TileContext(nc) as tc:
        with (
            tc.tile_pool(name="xpool", bufs=n_ld + 2) as xpool,
            tc.tile_pool(name="wpool", bufs=6) as wpool,
            tc.tile_pool(name="opool", bufs=4) as opool,
            tc.tile_pool(name="pspool", bufs=8, space="PSUM") as pspool,
        ):
            for rep, mb in ((r_, m_) for r_ in range(reps) for m_ in range(n_mb)):
                xts = []
                for i in range(n_ld):
                    xt = xpool.tile([P, KO_LD, m_block], fp8, tag="xt",
                                    name=f"x_{rep}_{mb}_{i}")
                    nc.sync.dma_start(xt, xb[mb, :, i * KO_LD:(i + 1) * KO_LD, :])
                    xts.append(xt)
                m0 = mb * m_block
                for nc0 in range(n_nc):
                    c0 = nc0 * n_chunk
                    psums = [
                        pspool.tile([P, MM_N], f32, tag="ps",
                                    name=f"ps_{rep}_{mb}_{nc0}_{i}")
                        for i in range(ms_n * nt_n)
                    ]
                    for kp in range(kp_n):
                        wt = wpool.tile([P, 2, n_chunk], fp8, tag="wt")
                        nc.sync.dma_start(wt, wb[:, 2 * kp:2 * kp + 2,
                                                 c0:c0 + n_chunk])
                        first = kp == 0
                        last = kp == kp_n - 1
                        xt = xts[(2 * kp) // KO_LD]
                        kj = (2 * kp) % KO_LD
                        for ms in range(ms_n):
                            lhs = xt[:, kj:kj + 2, ms * P:(ms + 1) * P]
                            for nt in range(nt_n):
                                nc.tensor.matmul(
                                    psums[ms * nt_n + nt],
                                    lhs,
                                    wt[:, :, nt * MM_N:(nt + 1) * MM_N],
                                    start=first,
                                    stop=last,
                                    perf_mode=dr,
                                )
                    for ms in range(ms_n):
                        for nt in range(nt_n):
                            st = opool.tile([P, MM_N], f32, tag="st")
                            nc.vector.tensor_copy(out=st, in_=psums[ms * nt_n + nt])
                            nc.sync.dma_start(
                                out[m0 + ms * P:m0 + (ms + 1) * P,
                                    c0 + nt * MM_N:c0 + (nt + 1) * MM_N],
                                st,
                            )
    nc.compile()
    return nc


def _build_bf1_nomm(k, m_core, n, m_block=512, n_chunk=1024, reps=1):
    """Diagnostic: bf1's exact DMA stream with no matmuls (times pure DMA)."""
    import concourse.mybir as mybir
    import concourse.tile as tile
    from concourse import bacc

    ko_n = k // P
    n_mb = m_core // m_block
    n_nc = n // n_chunk
    n_ld = ko_n // KO_LD

    nc = bacc.Bacc(None, target_bir_lowering=False, debug=False)
    xb = nc.declare_dram_parameter("xb", [n_mb, P, ko_n, m_block],
                                   mybir.dt.bfloat16, isOutput=False)
    wb = nc.declare_dram_parameter("wb", [P, ko_n, n],
                                   mybir.dt.bfloat16, isOutput=False)
    out = nc.declare_dram_parameter("out", [m_core, n], mybir.dt.float32,
                                    isOutput=True)
    bf16 = mybir.dt.bfloat16
    with tile.TileContext(nc) as tc:
        with (
            tc.tile_pool(name="xpool", bufs=n_ld + 2) as xpool,
            tc.tile_pool(name="wpool", bufs=6) as wpool,
        ):
            for rep, mb in ((r_, m_) for r_ in range(reps) for m_ in range(n_mb)):
                for i in range(n_ld):
                    xt = xpool.tile([P, KO_LD, m_block], bf16, tag="xt",
                                    name=f"x_{rep}_{mb}_{i}")
                    nc.sync.dma_start(xt, xb[mb, :, i * KO_LD:(i + 1) * KO_LD, :])
                for nc0 in range(n_nc):
                    c0 = nc0 * n_chunk
                    for ko in range(ko_n):
                        wt = wpool.tile([P, n_chunk], bf16, tag="wt")
                        nc.sync.dma_start(wt, wb[:, ko, c0:c0 + n_chunk])
    nc.compile()
    return nc


def _build_bf1_nodma(k, m_core, n, m_block=512, n_chunk=1024, reps=1):
    """Diagnostic: bf1's exact matmul stream with W loaded once (times pure PE)."""
    import concourse.mybir as mybir
    import concourse.tile as tile
    from concourse import bacc

    ko_n = k // P
    n_mb = m_core // m_block
    n_nc = n // n_chunk
    ms_n = m_block // P
    nt_n = n_chunk // MM_N
    n_ld = ko_n // KO_LD

    nc = bacc.Bacc(None, target_bir_lowering=False, debug=False)
    xb = nc.declare_dram_parameter("xb", [n_mb, P, ko_n, m_block],
                                   mybir.dt.bfloat16, isOutput=False)
    wb = nc.declare_dram_parameter("wb", [P, ko_n, n],
                                   mybir.dt.bfloat16, isOutput=False)
    out = nc.declare_dram_parameter("out", [m_core, n], mybir.dt.float32,
                                    isOutput=True)
    f32 = mybir.dt.float32
    bf16 = mybir.dt.bfloat16
    with tile.TileContext(nc) as tc:
        with (
            tc.tile_pool(name="xpool", bufs=2) as xpool,
            tc.tile_pool(name="wpool", bufs=1) as wpool,
            tc.tile_pool(name="opool", bufs=4) as opool,
            tc.tile_pool(name="pspool", bufs=8, space="PSUM") as pspool,
        ):
            xt = xpool.tile([P, KO_LD, m_block], bf16, tag="xt")
            nc.sync.dma_start(xt, xb[0, :, 0:KO_LD, :])
            wt = wpool.tile([P, n_chunk], bf16, tag="wt")
            nc.sync.dma_start(wt, wb[:, 0, 0:n_chunk])
            for rep, mb in ((r_, m_) for r_ in range(reps) for m_ in range(n_mb)):
                m0 = mb * m_block
                for nc0 in range(n_nc):
                    c0 = nc0 * n_chunk
                    psums = [
                        pspool.tile([P, MM_N], f32, tag="ps",
                                    name=f"ps_{rep}_{mb}_{nc0}_{i}")
                        for i in range(ms_n * nt_n)
                    ]
                    for ko in range(ko_n):
                        first = ko == 0
                        last = ko == ko_n - 1
                        kj = ko % KO_LD
                        for ms in range(ms_n):
                            lhs = xt[:, kj, ms * P:(ms + 1) * P]
                            for nt in range(nt_n):
                                nc.tensor.matmul(
                                    psums[ms * nt_n + nt],
                                    lhs,
                                    wt[:, nt * MM_N:(nt + 1) * MM_N],
                                    start=first,
                                    stop=last,
                                )
                    for ms in range(ms_n):
                        for nt in range(nt_n):
                            st = opool.tile([P, MM_N], f32, tag="st")
                            nc.vector.tensor_copy(out=st, in_=psums[ms * nt_n + nt])
                            nc.sync.dma_start(
                                out[m0 + ms * P:m0 + (ms + 1) * P,
                                    c0 + nt * MM_N:c0 + (nt + 1) * MM_N],
                                st,
                            )
    nc.compile()
    return nc


_BUILDERS = {
    "bf1": _build_bf1,
    "fp8": _build_fp8,
    "bf1_nomm": _build_bf1_nomm,
    "bf1_nodma": _build_bf1_nodma,
}


def _variant():
    return os.environ.get("KERNEL_VARIANT", "bf1")


def _get_nc(k, m_core, n, **kw):
    variant = _variant()
    key = (variant, k, m_core, n, tuple(sorted(kw.items())))
    if key not in _BUILD_CACHE:
        _BUILD_CACHE[key] = _BUILDERS[variant](k, m_core, n, **kw)
    return _BUILD_CACHE[key]


def _to_pkm_blocks(a, m_block, dtype):
    """[rows, k] fp32 -> contiguous [n_mb, P, ko_n, m_block] in `dtype`
    (k = ko*128 + p)."""
    rows, k = a.shape
    n_mb = rows // m_block
    ko_n = k // P
    a = a.astype(dtype)
    a = a.reshape(n_mb, m_block, ko_n, P).transpose(0, 3, 2, 1)
    return np.ascontiguousarray(a)


def _w_to_pkn(w, dtype, scale=1.0):
    """[n, k] fp32 -> contiguous [P, ko_n, n] in `dtype`."""
    n, k = w.shape
    ko_n = k // P
    if scale != 1.0:
        w = w * np.float32(scale)
    w = w.astype(dtype)
    w = w.reshape(n, ko_n, P).transpose(2, 1, 0)
    return np.ascontiguousarray(w)


def _make_runner(nc):
    """Build the sharded PJRT executor for `nc` across the 8 cores.

    Mirrors concourse.bass2jax.run_bass_via_pjrt, but returns a reusable
    closure so repeated calls share one jit cache and inputs can stay
    device-resident for benchmarking.
    """
    import jax
    import concourse.mybir as mybir
    from concourse import bass2jax
    from jax.experimental.shard_map import shard_map
    from jax.sharding import Mesh, NamedSharding, PartitionSpec

    bass2jax.install_neuronx_cc_hook()

    partition_name = nc.partition_id_tensor.name if nc.partition_id_tensor else None
    assert nc.dbg_addr is None

    in_names, out_names, out_avals = [], [], []
    for alloc in nc.m.functions[0].allocations:
        if not isinstance(alloc, mybir.MemoryLocationSet):
            continue
        name = alloc.memorylocations[0].name
        if alloc.kind == "ExternalInput":
            if name != partition_name:
                in_names.append(name)
        elif alloc.kind == "ExternalOutput":
            out_names.append(name)
            out_avals.append(
                jax.core.ShapedArray(tuple(alloc.tensor_shape), mybir.dt.np(alloc.dtype))
            )
    n_params = len(in_names)
    n_outs = len(out_avals)
    all_in_names = tuple(in_names) + tuple(out_names)
    if partition_name is not None:
        all_in_names = all_in_names + (partition_name,)
    donate = tuple(range(n_params, n_params + n_outs))

    def _body(*args):
        operands = list(args)
        if partition_name is not None:
            operands.append(bass2jax.partition_id_tensor())
        outs = bass2jax._bass_exec_p.bind(
            *operands,
            out_avals=tuple(out_avals),
            in_names=all_in_names,
            out_names=tuple(out_names),
            lowering_input_output_aliases=(),
            sim_require_finite=True,
            sim_require_nnan=True,
            nc=nc,
        )
        return tuple(outs)

    devices = jax.devices()[:N_CORES]
    assert len(devices) == N_CORES
    mesh = Mesh(np.asarray(devices), ("core",))
    spec = PartitionSpec("core")
    sharded = jax.jit(
        shard_map(
            _body,
            mesh=mesh,
            in_specs=(spec,) * (n_params + n_outs),
            out_specs=(spec,) * n_outs,
            check_rep=False,
        ),
        donate_argnums=donate,
        keep_unused=True,
    )
    sharding = NamedSharding(mesh, spec)
    return {
        "sharded": sharded,
        "sharding": sharding,
        "in_names": in_names,
        "out_names": out_names,
        "out_avals": out_avals,
        "n_params": n_params,
        "n_outs": n_outs,
    }


def _get_runner(nc):
    key = id(nc)
    if key not in _RUNNER_CACHE:
        _RUNNER_CACHE[key] = _make_runner(nc)
    return _RUNNER_CACHE[key]


def _run(nc, in_maps):
    """Execute the kernel across 8 cores; returns per-core output dicts."""
    import numpy as np

    r = _get_runner(nc)
    n_cores = len(in_maps)
    concat_in = [
        np.concatenate([np.asarray(m[name]) for m in in_maps], axis=0)
        for name in r["in_names"]
    ]
    concat_zeros = [
        np.zeros((n_cores * a.shape[0], *a.shape[1:]), a.dtype) for a in r["out_avals"]
    ]
    out_arrs = r["sharded"](*concat_in, *concat_zeros)
    return [
        {
            name: np.asarray(out_arrs[i]).reshape(n_cores, *r["out_avals"][i].shape)[c]
            for i, name in enumerate(r["out_names"])
        }
        for c in range(n_cores)
    ]


def _bench(in_maps, k, m_core, n, reps):
    """Measure steady-state per-GEMM time: the kernel repeated `reps` times
    inside one program, minus the reps=1 program, divided by reps-1. Fixed
    dispatch overhead cancels in the difference. Sets LAST_EXEC_NS."""
    global LAST_EXEC_NS
    import time

    import jax
    import jax.numpy as jnp
    import numpy as np

    times = {}
    dev_in = None
    for r_reps in (1, reps):
        nc = _get_nc(k, m_core, n, reps=r_reps)
        r = _get_runner(nc)
        if dev_in is None:
            concat_in = [
                np.concatenate([np.asarray(m[name]) for m in in_maps], axis=0)
                for name in r["in_names"]
            ]
            dev_in = [jax.device_put(a, r["sharding"]) for a in concat_in]
            jax.block_until_ready(dev_in)

        def _zeros():
            zs = [
                jax.jit(lambda a=a: jnp.zeros(a.shape, a.dtype),
                        out_shardings=r["sharding"])()
                for a in r["out_avals"]
            ]
            jax.block_until_ready(zs)
            return zs

        out = r["sharded"](*dev_in, *_zeros())  # compile + warmup
        jax.block_until_ready(out)
        attempts = []
        for _ in range(int(os.environ.get("KERNEL_BENCH_TRIES", "5"))):
            zs = _zeros()
            t0 = time.perf_counter()
            out = r["sharded"](*dev_in, *zs)
            jax.block_until_ready(out)
            attempts.append(time.perf_counter() - t0)
        times[r_reps] = min(attempts)
        print(f"[bench] reps={r_reps}: best {min(attempts) * 1e3:.3f} ms  "
              f"all {[f'{a * 1e3:.2f}' for a in attempts]}")

    per_iter = (times[reps] - times[1]) / (reps - 1)
    LAST_EXEC_NS = int(per_iter * 1e9)
    print(f"[bench] per-GEMM: {per_iter * 1e3:.3f} ms "
          f"(fixed+1iter: {times[1] * 1e3:.3f} ms)")


def kernel(input_, weight, bias):
    global LAST_RESULTS

    input_ = np.asarray(input_, dtype=np.float32)
    weight = np.asarray(weight, dtype=np.float32)
    bias = np.asarray(bias, dtype=np.float32)

    seq, batch, k = input_.shape
    n = weight.shape[0]
    m_full = seq * batch
    m_core = m_full // N_CORES

    variant = _variant()
    nc = _get_nc(k, m_core, n)

    x2 = input_.reshape(m_full, k)
    if variant == "fp8":
        m_block, dtype, w_scale = 256, E4M3, W_SCALE
    else:
        m_block, dtype, w_scale = 512, BF16, 1.0
    wp = _w_to_pkn(weight, dtype, scale=w_scale)

    in_maps = []
    for c in range(N_CORES):
        xp = _to_pkm_blocks(x2[c * m_core:(c + 1) * m_core], m_block, dtype)
        in_maps.append({"xb": xp, "wb": wp})

    results = _run(nc, in_maps)
    LAST_RESULTS = results

    bench_reps = int(os.environ.get("KERNEL_BENCH", "0"))
    if bench_reps > 1:
        _bench(in_maps, k, m_core, n, bench_reps)

    out = np.concatenate([results[c]["out"] for c in range(N_CORES)], axis=0)
    if w_scale != 1.0:
        out = out * np.float32(1.0 / w_scale)
    out = out.reshape(seq, batch, n)
    if bias.any():
        out = out + bias
    return out


# revision 7
# speedup vs baseline: 4.1334x; 1.3798x over previous
"""Trainium2 Bass kernel: row-parallel linear  y = einsum('sbk,nk->sbn', x, W) + bias.

Strategy
--------
Full inputs arrive on the host. We flatten (seq, batch) -> M = 8192 rows and
shard M across the 8 NeuronCores (1024 rows each); every core streams the full
weight and computes its [1024, 4096] slice of the output.

The correctness gate is rel_err < 2e-2 (max-abs over max-abs), which a single
reduced-precision GEMM pass meets comfortably:
  - "bf1": one bf16 pass              (measured ~9e-4 rel err)
  - "fp8": one e4m3 DoubleRow pass    (measured ~1.5e-2 rel err) at ~1.5-1.8x
    the bf16 matmul rate (256-deep contraction per PE instruction).
Host-side quantization makes the device error deterministic: products are
exact in fp8/bf16 and accumulate in fp32 PSUM.

Device layout: operands are staged in DRAM with the contraction dim on the
partition axis: x as [n_mb, P, ko, m_block] and W as [P, ko, n] with
k = ko*128 + p, so every SBUF tile load is contiguous-per-partition.

Per core: loop over m-blocks; per m-block the full-K x strip stays resident
in SBUF (loaded as ko-chunked tiles so matmuls start as soon as their chunk
lands and the next block prefetches into spare pool slots); W streams through
once per m-block; 8 PSUM banks hold the (m-strip x n-tile) accumulators
across the whole K loop, evicted once per n-chunk via VectorE.
"""

import os

import numpy as np
import ml_dtypes

BF16 = ml_dtypes.bfloat16
E4M3 = ml_dtypes.float8_e4m3  # TRN semantics: max normal +-240

# Problem shapes (hardcoded per contest contract).
SEQ, BATCH, D_FF, D_MODEL = 2048, 4, 16384, 4096
N_CORES = 8
P = 128

M_FULL = SEQ * BATCH            # 8192
M_CORE = M_FULL // N_CORES      # 1024

MM_N = 512                      # matmul free dim (one fp32 PSUM bank)
KO_LD = 16                      # ko chunks per x load tile

W_SCALE = 128.0                 # fp8: weight pre-scale (power of two, exact)

# Exec-time of the last hardware benchmark (ns), populated when KERNEL_BENCH>0.
LAST_EXEC_NS = None
LAST_RESULTS = None

_BUILD_CACHE = {}
_RUNNER_CACHE = {}


def _build_bf1(k, m_core, n, m_block=512, n_chunk=1024, w_ld=None, reps=1):
    """Single-pass bf16 GEMM: out[m_core, n] = x[m_core, k] @ w[n, k]^T.

    PSUM holds (m_block/128) x (n_chunk/512) fp32 accumulators across the
    full K loop; consecutive matmuls rotate banks. W is streamed once per
    m-block in [P, w_ld, n_chunk] tiles (per-dma_start fixed cost ~1.5us
    dominates below ~1MB transfers, so batch ko planes per DMA); x tiles
    are ko-chunked for fine-grained deps + prefetch; evictions are paired
    into one 512KB output DMA per psum pair."""
    import concourse.mybir as mybir
    import concourse.tile as tile
    from concourse import bacc

    if w_ld is None:
        w_ld = int(os.environ.get("KERNEL_WLD", "4"))
    ko_n = k // P
    n_mb = m_core // m_block
    n_nc = n // n_chunk
    ms_n = m_block // P
    nt_n = n_chunk // MM_N
    n_ld = ko_n // KO_LD
    assert ms_n * nt_n <= 8, "PSUM banks exceeded"
    assert KO_LD % w_ld == 0

    nc = bacc.Bacc(None, target_bir_lowering=False, debug=False)
    xb = nc.declare_dram_parameter("xb", [n_mb, P, ko_n, m_block],
                                   mybir.dt.bfloat16, isOutput=False)
    wb = nc.declare_dram_parameter("wb", [P, ko_n, n],
                                   mybir.dt.bfloat16, isOutput=False)
    out = nc.declare_dram_parameter("out", [m_core, n], mybir.dt.float32,
                                    isOutput=True)

    f32 = mybir.dt.float32
    bf16 = mybir.dt.bfloat16

    with tile.TileContext(nc) as tc:
        with (
            tc.tile_pool(name="xpool", bufs=n_ld + 1) as xpool,
            tc.tile_pool(name="wpool", bufs=4) as wpool,
            tc.tile_pool(name="opool", bufs=2) as opool,
            tc.tile_pool(name="pspool", bufs=8, space="PSUM") as pspool,
        ):
            for rep, mb in ((r_, m_) for r_ in range(reps) for m_ in range(n_mb)):
                xts = []
                for i in range(n_ld):
                    xt = xpool.tile([P, KO_LD, m_block], bf16, tag="xt",
                                    name=f"x_{rep}_{mb}_{i}")
                    nc.sync.dma_start(xt, xb[mb, :, i * KO_LD:(i + 1) * KO_LD, :])
                    xts.append(xt)
                m0 = mb * m_block
                for nc0 in range(n_nc):
                    c0 = nc0 * n_chunk
                    psums = [
                        pspool.tile([P, MM_N], f32, tag="ps",
                                    name=f"ps_{rep}_{mb}_{nc0}_{i}")
                        for i in range(ms_n * nt_n)
                    ]
                    for kw in range(ko_n // w_ld):
                        wt = wpool.tile([P, w_ld, n_chunk], bf16, tag="wt")
                        nc.sync.dma_start(
                            wt, wb[:, kw * w_ld:(kw + 1) * w_ld, c0:c0 + n_chunk])
                        for kj in range(w_ld):
                            ko = kw * w_ld + kj
                            first = ko == 0
                            last = ko == ko_n - 1
                            xt = xts[ko // KO_LD]
                            for ms in range(ms_n):
                                lhs = xt[:, ko % KO_LD, ms * P:(ms + 1) * P]
                                for nt in range(nt_n):
                                    nc.tensor.matmul(
                                        psums[ms * nt_n + nt],
                                        lhs,
                                        wt[:, kj, nt * MM_N:(nt + 1) * MM_N],
                                        start=first,
                                        stop=last,
                                    )
                    for ms in range(ms_n):
                        st = opool.tile([P, nt_n * MM_N], f32, tag="st")
                        for nt in range(nt_n):
                            nc.vector.tensor_copy(
                                out=st[:, nt * MM_N:(nt + 1) * MM_N],
                                in_=psums[ms * nt_n + nt])
                        nc.sync.dma_start(
                            out[m0 + ms * P:m0 + (ms + 1) * P,
                                c0:c0 + nt_n * MM_N],
                            st,
                        )
    nc.compile()
    return nc


def _build_fp8(k, m_core, n, m_block=256, n_chunk=2048, w_ld=None, reps=1):
    """Single-pass e4m3 GEMM with DoubleRow: each matmul contracts 256 rows
    (2 ko chunks packed per PE cell). Both operands carry a [P, 2, free] AP.
    W arrives pre-scaled by W_SCALE; the host descales the output. W is
    streamed in [P, w_ld, n_chunk] tiles to amortize per-DMA fixed cost."""
    import concourse.mybir as mybir
    import concourse.tile as tile
    from concourse import bacc

    if w_ld is None:
        w_ld = int(os.environ.get("KERNEL_WLD", "4"))
    ko_n = k // P
    n_mb = m_core // m_block
    n_nc = n // n_chunk
    ms_n = m_block // P
    nt_n = n_chunk // MM_N
    n_ld = ko_n // KO_LD
    assert ms_n * nt_n <= 8, "PSUM banks exceeded"
    assert w_ld % 2 == 0 and KO_LD % w_ld == 0

    nc = bacc.Bacc(None, target_bir_lowering=False, debug=False)
    xb = nc.declare_dram_parameter("xb", [n_mb, P, ko_n, m_block],
                                   mybir.dt.float8e4, isOutput=False)
    wb = nc.declare_dram_parameter("wb", [P, ko_n, n],
                                   mybir.dt.float8e4, isOutput=False)
    out = nc.declare_dram_parameter("out", [m_core, n], mybir.dt.float32,
                                    isOutput=True)

    f32 = mybir.dt.float32
    fp8 = mybir.dt.float8e4
    dr = mybir.MatmulPerfMode.DoubleRow

    with tile.TileContext(nc) as tc:
        with (
            tc.tile_pool(name="xpool", bufs=n_ld + 2) as xpool,
            tc.tile_pool(name="wpool", bufs=4) as wpool,
            tc.tile_pool(name="opool", bufs=2) as opool,
            tc.tile_pool(name="pspool", bufs=8, space="PSUM") as pspool,
        ):
            for rep, mb in ((r_, m_) for r_ in range(reps) for m_ in range(n_mb)):
                xts = []
                for i in range(n_ld):
                    xt = xpool.tile([P, KO_LD, m_block], fp8, tag="xt",
                                    name=f"x_{rep}_{mb}_{i}")
                    nc.sync.dma_start(xt, xb[mb, :, i * KO_LD:(i + 1) * KO_LD, :])
                    xts.append(xt)
                m0 = mb * m_block
                for nc0 in range(n_nc):
                    c0 = nc0 * n_chunk
                    psums = [
                        pspool.tile([P, MM_N], f32, tag="ps",
                                    name=f"ps_{rep}_{mb}_{nc0}_{i}")
                        for i in range(ms_n * nt_n)
                    ]
                    for kw in range(ko_n // w_ld):
                        wt = wpool.tile([P, w_ld, n_chunk], fp8, tag="wt")
                        nc.sync.dma_start(
                            wt, wb[:, kw * w_ld:(kw + 1) * w_ld, c0:c0 + n_chunk])
                        for kj in range(0, w_ld, 2):
                            ko = kw * w_ld + kj
                            first = ko == 0
                            last = ko == ko_n - 2
                            xt = xts[ko // KO_LD]
                            kx = ko % KO_LD
                            for ms in range(ms_n):
                                lhs = xt[:, kx:kx + 2, ms * P:(ms + 1) * P]
                                for nt in range(nt_n):
                                    nc.tensor.matmul(
                                        psums[ms * nt_n + nt],
                                        lhs,
                                        wt[:, kj:kj + 2,
                                           nt * MM_N:(nt + 1) * MM_N],
                                        start=first,
                                        stop=last,
                                        perf_mode=dr,
                                    )
                    for ms in range(ms_n):
                        st = opool.tile([P, nt_n * MM_N], f32, tag="st")
                        for nt in range(nt_n):
                            nc.vector.tensor_copy(
                                out=st[:, nt * MM_N:(nt + 1) * MM_N],
                                in_=psums[ms * nt_n + nt])
                        nc.sync.dma_start(
                            out[m0 + ms * P:m0 + (ms + 1) * P,
                                c0:c0 + nt_n * MM_N],
                            st,
                        )
    nc.compile()
    return nc


def _build_bf1_nomm(k, m_core, n, m_block=512, n_chunk=1024, reps=1):
    """Diagnostic: bf1's exact DMA stream with no matmuls (times pure DMA)."""
    import concourse.mybir as mybir
    import concourse.tile as tile
    from concourse import bacc

    ko_n = k // P
    n_mb = m_core // m_block
    n_nc = n // n_chunk
    n_ld = ko_n // KO_LD

    nc = bacc.Bacc(None, target_bir_lowering=False, debug=False)
    xb = nc.declare_dram_parameter("xb", [n_mb, P, ko_n, m_block],
                                   mybir.dt.bfloat16, isOutput=False)
    wb = nc.declare_dram_parameter("wb", [P, ko_n, n],
                                   mybir.dt.bfloat16, isOutput=False)
    out = nc.declare_dram_parameter("out", [m_core, n], mybir.dt.float32,
                                    isOutput=True)
    bf16 = mybir.dt.bfloat16
    with tile.TileContext(nc) as tc:
        with (
            tc.tile_pool(name="xpool", bufs=n_ld + 2) as xpool,
            tc.tile_pool(name="wpool", bufs=6) as wpool,
        ):
            for rep, mb in ((r_, m_) for r_ in range(reps) for m_ in range(n_mb)):
                for i in range(n_ld):
                    xt = xpool.tile([P, KO_LD, m_block], bf16, tag="xt",
                                    name=f"x_{rep}_{mb}_{i}")
                    nc.sync.dma_start(xt, xb[mb, :, i * KO_LD:(i + 1) * KO_LD, :])
                for nc0 in range(n_nc):
                    c0 = nc0 * n_chunk
                    for ko in range(ko_n):
                        wt = wpool.tile([P, n_chunk], bf16, tag="wt")
                        nc.sync.dma_start(wt, wb[:, ko, c0:c0 + n_chunk])
    nc.compile()
    return nc


def _build_bf1_nodma(k, m_core, n, m_block=512, n_chunk=1024, reps=1):
    """Diagnostic: bf1's exact matmul stream with W loaded once (times pure PE)."""
    import concourse.mybir as mybir
    import concourse.tile as tile
    from concourse import bacc

    ko_n = k // P
    n_mb = m_core // m_block
    n_nc = n // n_chunk
    ms_n = m_block // P
    nt_n = n_chunk // MM_N
    n_ld = ko_n // KO_LD

    nc = bacc.Bacc(None, target_bir_lowering=False, debug=False)
    xb = nc.declare_dram_parameter("xb", [n_mb, P, ko_n, m_block],
                                   mybir.dt.bfloat16, isOutput=False)
    wb = nc.declare_dram_parameter("wb", [P, ko_n, n],
                                   mybir.dt.bfloat16, isOutput=False)
    out = nc.declare_dram_parameter("out", [m_core, n], mybir.dt.float32,
                                    isOutput=True)
    f32 = mybir.dt.float32
    bf16 = mybir.dt.bfloat16
    with tile.TileContext(nc) as tc:
        with (
            tc.tile_pool(name="xpool", bufs=2) as xpool,
            tc.tile_pool(name="wpool", bufs=1) as wpool,
            tc.tile_pool(name="opool", bufs=4) as opool,
            tc.tile_pool(name="pspool", bufs=8, space="PSUM") as pspool,
        ):
            xt = xpool.tile([P, KO_LD, m_block], bf16, tag="xt")
            nc.sync.dma_start(xt, xb[0, :, 0:KO_LD, :])
            wt = wpool.tile([P, n_chunk], bf16, tag="wt")
            nc.sync.dma_start(wt, wb[:, 0, 0:n_chunk])
            for rep, mb in ((r_, m_) for r_ in range(reps) for m_ in range(n_mb)):
                m0 = mb * m_block
                for nc0 in range(n_nc):
                    c0 = nc0 * n_chunk
                    psums = [
                        pspool.tile([P, MM_N], f32, tag="ps",
                                    name=f"ps_{rep}_{mb}_{nc0}_{i}")
                        for i in range(ms_n * nt_n)
                    ]
                    for ko in range(ko_n):
                        first = ko == 0
                        last = ko == ko_n - 1
                        kj = ko % KO_LD
                        for ms in range(ms_n):
                            lhs = xt[:, kj, ms * P:(ms + 1) * P]
                            for nt in range(nt_n):
                                nc.tensor.matmul(
                                    psums[ms * nt_n + nt],
                                    lhs,
                                    wt[:, nt * MM_N:(nt + 1) * MM_N],
                                    start=first,
                                    stop=last,
                                )
                    for ms in range(ms_n):
                        for nt in range(nt_n):
                            st = opool.tile([P, MM_N], f32, tag="st")
                            nc.vector.tensor_copy(out=st, in_=psums[ms * nt_n + nt])
                            nc.sync.dma_start(
                                out[m0 + ms * P:m0 + (ms + 1) * P,
                                    c0 + nt * MM_N:c0 + (nt + 1) * MM_N],
                                st,
                            )
    nc.compile()
    return nc


_BUILDERS = {
    "bf1": _build_bf1,
    "fp8": _build_fp8,
    "fp8b": lambda k, m, n, **kw: _build_fp8(k, m, n, m_block=512, n_chunk=1024,
                                             w_ld=8, **kw),
    "bf1_nomm": _build_bf1_nomm,
    "bf1_nodma": _build_bf1_nodma,
}

# variant -> (m_block for host x layout, operand dtype, W pre-scale)
VARIANT_CFG = {
    "bf1": (512, BF16, 1.0),
    "fp8": (256, E4M3, W_SCALE),
    "fp8b": (512, E4M3, W_SCALE),
    "bf1_nomm": (512, BF16, 1.0),
    "bf1_nodma": (512, BF16, 1.0),
}


def _variant():
    return os.environ.get("KERNEL_VARIANT", "bf1")


def _get_nc(k, m_core, n, **kw):
    variant = _variant()
    key = (variant, k, m_core, n, tuple(sorted(kw.items())))
    if key not in _BUILD_CACHE:
        _BUILD_CACHE[key] = _BUILDERS[variant](k, m_core, n, **kw)
    return _BUILD_CACHE[key]


def _to_pkm_blocks(a, m_block, dtype):
    """[rows, k] fp32 -> contiguous [n_mb, P, ko_n, m_block] in `dtype`
    (k = ko*128 + p)."""
    rows, k = a.shape
    n_mb = rows // m_block
    ko_n = k // P
    a = a.astype(dtype)
    a = a.reshape(n_mb, m_block, ko_n, P).transpose(0, 3, 2, 1)
    return np.ascontiguousarray(a)


def _w_to_pkn(w, dtype, scale=1.0):
    """[n, k] fp32 -> contiguous [P, ko_n, n] in `dtype`."""
    n, k = w.shape
    ko_n = k // P
    if scale != 1.0:
        w = w * np.float32(scale)
    w = w.astype(dtype)
    w = w.reshape(n, ko_n, P).transpose(2, 1, 0)
    return np.ascontiguousarray(w)


def _make_runner(nc):
    """Build the sharded PJRT executor for `nc` across the 8 cores.

    Mirrors concourse.bass2jax.run_bass_via_pjrt, but returns a reusable
    closure so repeated calls share one jit cache and inputs can stay
    device-resident for benchmarking.
    """
    import jax
    import concourse.mybir as mybir
    from concourse import bass2jax
    from jax.experimental.shard_map import shard_map
    from jax.sharding import Mesh, NamedSharding, PartitionSpec

    bass2jax.install_neuronx_cc_hook()

    partition_name = nc.partition_id_tensor.name if nc.partition_id_tensor else None
    assert nc.dbg_addr is None

    in_names, out_names, out_avals = [], [], []
    for alloc in nc.m.functions[0].allocations:
        if not isinstance(alloc, mybir.MemoryLocationSet):
            continue
        name = alloc.memorylocations[0].name
        if alloc.kind == "ExternalInput":
            if name != partition_name:
                in_names.append(name)
        elif alloc.kind == "ExternalOutput":
            out_names.append(name)
            out_avals.append(
                jax.core.ShapedArray(tuple(alloc.tensor_shape), mybir.dt.np(alloc.dtype))
            )
    n_params = len(in_names)
    n_outs = len(out_avals)
    all_in_names = tuple(in_names) + tuple(out_names)
    if partition_name is not None:
        all_in_names = all_in_names + (partition_name,)
    donate = tuple(range(n_params, n_params + n_outs))

    def _body(*args):
        operands = list(args)
        if partition_name is not None:
            operands.append(bass2jax.partition_id_tensor())
        outs = bass2jax._bass_exec_p.bind(
            *operands,
            out_avals=tuple(out_avals),
            in_names=all_in_names,
            out_names=tuple(out_names),
            lowering_input_output_aliases=(),
            sim_require_finite=True,
            sim_require_nnan=True,
            nc=nc,
        )
        return tuple(outs)

    devices = jax.devices()[:N_CORES]
    assert len(devices) == N_CORES
    mesh = Mesh(np.asarray(devices), ("core",))
    spec = PartitionSpec("core")
    sharded = jax.jit(
        shard_map(
            _body,
            mesh=mesh,
            in_specs=(spec,) * (n_params + n_outs),
            out_specs=(spec,) * n_outs,
            check_rep=False,
        ),
        donate_argnums=donate,
        keep_unused=True,
    )
    sharding = NamedSharding(mesh, spec)
    return {
        "sharded": sharded,
        "sharding": sharding,
        "in_names": in_names,
        "out_names": out_names,
        "out_avals": out_avals,
        "n_params": n_params,
        "n_outs": n_outs,
    }


def _get_runner(nc):
    key = id(nc)
    if key not in _RUNNER_CACHE:
        _RUNNER_CACHE[key] = _make_runner(nc)
    return _RUNNER_CACHE[key]


def _run(nc, in_maps):
    """Execute the kernel across 8 cores; returns per-core output dicts."""
    import numpy as np

    r = _get_runner(nc)
    n_cores = len(in_maps)
    concat_in = [
        np.concatenate([np.asarray(m[name]) for m in in_maps], axis=0)
        for name in r["in_names"]
    ]
    concat_zeros = [
        np.zeros((n_cores * a.shape[0], *a.shape[1:]), a.dtype) for a in r["out_avals"]
    ]
    out_arrs = r["sharded"](*concat_in, *concat_zeros)
    return [
        {
            name: np.asarray(out_arrs[i]).reshape(n_cores, *r["out_avals"][i].shape)[c]
            for i, name in enumerate(r["out_names"])
        }
        for c in range(n_cores)
    ]


def _bench(in_maps, k, m_core, n, reps):
    """Measure steady-state per-GEMM time: the kernel repeated `reps` times
    inside one program, minus the reps=1 program, divided by reps-1. Fixed
    dispatch overhead cancels in the difference. Sets LAST_EXEC_NS."""
    global LAST_EXEC_NS
    import time

    import jax
    import jax.numpy as jnp
    import numpy as np

    times = {}
    dev_in = None
    for r_reps in (1, reps):
        nc = _get_nc(k, m_core, n, reps=r_reps)
        r = _get_runner(nc)
        if dev_in is None:
            concat_in = [
                np.concatenate([np.asarray(m[name]) for m in in_maps], axis=0)
                for name in r["in_names"]
            ]
            dev_in = [jax.device_put(a, r["sharding"]) for a in concat_in]
            jax.block_until_ready(dev_in)

        def _zeros():
            zs = [
                jax.jit(lambda a=a: jnp.zeros(a.shape, a.dtype),
                        out_shardings=r["sharding"])()
                for a in r["out_avals"]
            ]
            jax.block_until_ready(zs)
            return zs

        out = r["sharded"](*dev_in, *_zeros())  # compile + warmup
        jax.block_until_ready(out)
        attempts = []
        for _ in range(int(os.environ.get("KERNEL_BENCH_TRIES", "5"))):
            zs = _zeros()
            t0 = time.perf_counter()
            out = r["sharded"](*dev_in, *zs)
            jax.block_until_ready(out)
            attempts.append(time.perf_counter() - t0)
        times[r_reps] = min(attempts)
        print(f"[bench] reps={r_reps}: best {min(attempts) * 1e3:.3f} ms  "
              f"all {[f'{a * 1e3:.2f}' for a in attempts]}")

    per_iter = (times[reps] - times[1]) / (reps - 1)
    LAST_EXEC_NS = int(per_iter * 1e9)
    print(f"[bench] per-GEMM: {per_iter * 1e3:.3f} ms "
          f"(fixed+1iter: {times[1] * 1e3:.3f} ms)")


def kernel(input_, weight, bias):
    global LAST_RESULTS

    input_ = np.asarray(input_, dtype=np.float32)
    weight = np.asarray(weight, dtype=np.float32)
    bias = np.asarray(bias, dtype=np.float32)

    seq, batch, k = input_.shape
    n = weight.shape[0]
    m_full = seq * batch
    m_core = m_full // N_CORES

    variant = _variant()
    nc = _get_nc(k, m_core, n)

    x2 = input_.reshape(m_full, k)
    m_block, dtype, w_scale = VARIANT_CFG[variant]
    wp = _w_to_pkn(weight, dtype, scale=w_scale)

    in_maps = []
    for c in range(N_CORES):
        xp = _to_pkm_blocks(x2[c * m_core:(c + 1) * m_core], m_block, dtype)
        in_maps.append({"xb": xp, "wb": wp})

    results = _run(nc, in_maps)
    LAST_RESULTS = results

    bench_reps = int(os.environ.get("KERNEL_BENCH", "0"))
    if bench_reps > 1:
        _bench(in_maps, k, m_core, n, bench_reps)

    out = np.concatenate([results[c]["out"] for c in range(N_CORES)], axis=0)
    if w_scale != 1.0:
        out = out * np.float32(1.0 / w_scale)
    out = out.reshape(seq, batch, n)
    if bias.any():
        out = out + bias
    return out


# revision 9
# speedup vs baseline: 7.8386x; 1.8964x over previous
"""Trainium2 Bass kernel: row-parallel linear  y = einsum('sbk,nk->sbn', x, W) + bias.

Strategy
--------
Full inputs arrive on the host. We flatten (seq, batch) -> M = 8192 rows and
shard M across the 8 NeuronCores (1024 rows each); every core streams the full
weight and computes its [1024, 4096] slice of the output.

The correctness gate is rel_err < 2e-2 (max-abs over max-abs), which a single
reduced-precision GEMM pass meets comfortably:
  - "bf1": one bf16 pass              (measured ~9e-4 rel err)
  - "fp8": one e4m3 DoubleRow pass    (measured ~1.5e-2 rel err) at ~1.5-1.8x
    the bf16 matmul rate (256-deep contraction per PE instruction).
Host-side quantization makes the device error deterministic: products are
exact in fp8/bf16 and accumulate in fp32 PSUM.

Device layout: operands are staged in DRAM with the contraction dim on the
partition axis: x as [n_mb, P, ko, m_block] and W as [P, ko, n] with
k = ko*128 + p, so every SBUF tile load is contiguous-per-partition.

Per core: loop over m-blocks; per m-block the full-K x strip stays resident
in SBUF (loaded as ko-chunked tiles so matmuls start as soon as their chunk
lands and the next block prefetches into spare pool slots); W streams through
once per m-block; 8 PSUM banks hold the (m-strip x n-tile) accumulators
across the whole K loop, evicted once per n-chunk via VectorE.
"""

import os

import numpy as np
import ml_dtypes

BF16 = ml_dtypes.bfloat16
E4M3 = ml_dtypes.float8_e4m3  # TRN semantics: max normal +-240

# Problem shapes (hardcoded per contest contract).
SEQ, BATCH, D_FF, D_MODEL = 2048, 4, 16384, 4096
N_CORES = 8
P = 128

M_FULL = SEQ * BATCH            # 8192
M_CORE = M_FULL // N_CORES      # 1024

MM_N = 512                      # matmul free dim (one fp32 PSUM bank)
KO_LD = 16                      # ko chunks per x load tile

W_SCALE = 128.0                 # fp8: weight pre-scale (power of two, exact)

# Exec-time of the last hardware benchmark (ns), populated when KERNEL_BENCH>0.
LAST_EXEC_NS = None
LAST_RESULTS = None

_BUILD_CACHE = {}
_RUNNER_CACHE = {}


def _build_bf1(k, m_core, n, m_block=512, n_chunk=1024, w_ld=None, reps=1):
    """Single-pass bf16 GEMM: out[m_core, n] = x[m_core, k] @ w[n, k]^T.

    PSUM holds (m_block/128) x (n_chunk/512) fp32 accumulators across the
    full K loop; consecutive matmuls rotate banks. W is streamed once per
    m-block in [P, w_ld, n_chunk] tiles (per-dma_start fixed cost ~1.5us
    dominates below ~1MB transfers, so batch ko planes per DMA); x tiles
    are ko-chunked for fine-grained deps + prefetch; evictions are paired
    into one 512KB output DMA per psum pair."""
    import concourse.mybir as mybir
    import concourse.tile as tile
    from concourse import bacc

    if w_ld is None:
        w_ld = int(os.environ.get("KERNEL_WLD", "4"))
    ko_n = k // P
    n_mb = m_core // m_block
    n_nc = n // n_chunk
    ms_n = m_block // P
    nt_n = n_chunk // MM_N
    n_ld = ko_n // KO_LD
    assert ms_n * nt_n <= 8, "PSUM banks exceeded"
    assert KO_LD % w_ld == 0

    nc = bacc.Bacc(None, target_bir_lowering=False, debug=False)
    xb = nc.declare_dram_parameter("xb", [n_mb, P, ko_n, m_block],
                                   mybir.dt.bfloat16, isOutput=False)
    wb = nc.declare_dram_parameter("wb", [P, ko_n, n],
                                   mybir.dt.bfloat16, isOutput=False)
    out = nc.declare_dram_parameter("out", [m_core, n], mybir.dt.float32,
                                    isOutput=True)

    f32 = mybir.dt.float32
    bf16 = mybir.dt.bfloat16

    with tile.TileContext(nc) as tc:
        with (
            tc.tile_pool(name="xpool", bufs=n_ld + 1) as xpool,
            tc.tile_pool(name="wpool", bufs=4) as wpool,
            tc.tile_pool(name="opool", bufs=2) as opool,
            tc.tile_pool(name="pspool", bufs=8, space="PSUM") as pspool,
        ):
            for rep, mb in ((r_, m_) for r_ in range(reps) for m_ in range(n_mb)):
                xts = []
                for i in range(n_ld):
                    xt = xpool.tile([P, KO_LD, m_block], bf16, tag="xt",
                                    name=f"x_{rep}_{mb}_{i}")
                    nc.sync.dma_start(xt, xb[mb, :, i * KO_LD:(i + 1) * KO_LD, :])
                    xts.append(xt)
                m0 = mb * m_block
                for nc0 in range(n_nc):
                    c0 = nc0 * n_chunk
                    psums = [
                        pspool.tile([P, MM_N], f32, tag="ps",
                                    name=f"ps_{rep}_{mb}_{nc0}_{i}")
                        for i in range(ms_n * nt_n)
                    ]
                    for kw in range(ko_n // w_ld):
                        wt = wpool.tile([P, w_ld, n_chunk], bf16, tag="wt")
                        nc.sync.dma_start(
                            wt, wb[:, kw * w_ld:(kw + 1) * w_ld, c0:c0 + n_chunk])
                        for kj in range(w_ld):
                            ko = kw * w_ld + kj
                            first = ko == 0
                            last = ko == ko_n - 1
                            xt = xts[ko // KO_LD]
                            for ms in range(ms_n):
                                lhs = xt[:, ko % KO_LD, ms * P:(ms + 1) * P]
                                for nt in range(nt_n):
                                    nc.tensor.matmul(
                                        psums[ms * nt_n + nt],
                                        lhs,
                                        wt[:, kj, nt * MM_N:(nt + 1) * MM_N],
                                        start=first,
                                        stop=last,
                                    )
                    for ms in range(ms_n):
                        st = opool.tile([P, nt_n * MM_N], f32, tag="st")
                        for nt in range(nt_n):
                            nc.vector.tensor_copy(
                                out=st[:, nt * MM_N:(nt + 1) * MM_N],
                                in_=psums[ms * nt_n + nt])
                        nc.sync.dma_start(
                            out[m0 + ms * P:m0 + (ms + 1) * P,
                                c0:c0 + nt_n * MM_N],
                            st,
                        )
    nc.compile()
    return nc


def _build_fp8(k, m_core, n, m_block=256, n_chunk=2048, w_ld=None, reps=1):
    """Single-pass e4m3 GEMM with DoubleRow: each matmul contracts 256 rows
    (2 ko chunks packed per PE cell). Both operands carry a [P, 2, free] AP.
    W arrives pre-scaled by W_SCALE; the host descales the output. W is
    streamed in [P, w_ld, n_chunk] tiles to amortize per-DMA fixed cost."""
    import concourse.mybir as mybir
    import concourse.tile as tile
    from concourse import bacc

    if w_ld is None:
        w_ld = int(os.environ.get("KERNEL_WLD", "4"))
    ko_n = k // P
    n_mb = m_core // m_block
    n_nc = n // n_chunk
    ms_n = m_block // P
    nt_n = n_chunk // MM_N
    n_ld = ko_n // KO_LD
    assert ms_n * nt_n <= 8, "PSUM banks exceeded"
    assert w_ld % 2 == 0 and KO_LD % w_ld == 0

    nc = bacc.Bacc(None, target_bir_lowering=False, debug=False)
    xb = nc.declare_dram_parameter("xb", [n_mb, P, ko_n, m_block],
                                   mybir.dt.float8e4, isOutput=False)
    wb = nc.declare_dram_parameter("wb", [P, ko_n, n],
                                   mybir.dt.float8e4, isOutput=False)
    out = nc.declare_dram_parameter("out", [m_core, n], mybir.dt.float32,
                                    isOutput=True)

    f32 = mybir.dt.float32
    fp8 = mybir.dt.float8e4
    dr = mybir.MatmulPerfMode.DoubleRow

    with tile.TileContext(nc) as tc:
        with (
            tc.tile_pool(name="xpool", bufs=n_ld + 2) as xpool,
            tc.tile_pool(name="wpool", bufs=4) as wpool,
            tc.tile_pool(name="opool", bufs=2) as opool,
            tc.tile_pool(name="pspool", bufs=8, space="PSUM") as pspool,
        ):
            for rep, mb in ((r_, m_) for r_ in range(reps) for m_ in range(n_mb)):
                xts = []
                for i in range(n_ld):
                    xt = xpool.tile([P, KO_LD, m_block], fp8, tag="xt",
                                    name=f"x_{rep}_{mb}_{i}")
                    nc.sync.dma_start(xt, xb[mb, :, i * KO_LD:(i + 1) * KO_LD, :])
                    xts.append(xt)
                m0 = mb * m_block
                for nc0 in range(n_nc):
                    c0 = nc0 * n_chunk
                    psums = [
                        pspool.tile([P, MM_N], f32, tag="ps",
                                    name=f"ps_{rep}_{mb}_{nc0}_{i}")
                        for i in range(ms_n * nt_n)
                    ]
                    for kw in range(ko_n // w_ld):
                        wt = wpool.tile([P, w_ld, n_chunk], fp8, tag="wt")
                        nc.sync.dma_start(
                            wt, wb[:, kw * w_ld:(kw + 1) * w_ld, c0:c0 + n_chunk])
                        for kj in range(0, w_ld, 2):
                            ko = kw * w_ld + kj
                            first = ko == 0
                            last = ko == ko_n - 2
                            xt = xts[ko // KO_LD]
                            kx = ko % KO_LD
                            for ms in range(ms_n):
                                lhs = xt[:, kx:kx + 2, ms * P:(ms + 1) * P]
                                for nt in range(nt_n):
                                    nc.tensor.matmul(
                                        psums[ms * nt_n + nt],
                                        lhs,
                                        wt[:, kj:kj + 2,
                                           nt * MM_N:(nt + 1) * MM_N],
                                        start=first,
                                        stop=last,
                                        perf_mode=dr,
                                    )
                    for ms in range(ms_n):
                        st = opool.tile([P, nt_n * MM_N], f32, tag="st")
                        for nt in range(nt_n):
                            nc.vector.tensor_copy(
                                out=st[:, nt * MM_N:(nt + 1) * MM_N],
                                in_=psums[ms * nt_n + nt])
                        nc.sync.dma_start(
                            out[m0 + ms * P:m0 + (ms + 1) * P,
                                c0:c0 + nt_n * MM_N],
                            st,
                        )
    nc.compile()
    return nc


def _build_bf1_nomm(k, m_core, n, m_block=512, n_chunk=1024, reps=1):
    """Diagnostic: bf1's exact DMA stream with no matmuls (times pure DMA)."""
    import concourse.mybir as mybir
    import concourse.tile as tile
    from concourse import bacc

    ko_n = k // P
    n_mb = m_core // m_block
    n_nc = n // n_chunk
    n_ld = ko_n // KO_LD

    nc = bacc.Bacc(None, target_bir_lowering=False, debug=False)
    xb = nc.declare_dram_parameter("xb", [n_mb, P, ko_n, m_block],
                                   mybir.dt.bfloat16, isOutput=False)
    wb = nc.declare_dram_parameter("wb", [P, ko_n, n],
                                   mybir.dt.bfloat16, isOutput=False)
    out = nc.declare_dram_parameter("out", [m_core, n], mybir.dt.float32,
                                    isOutput=True)
    bf16 = mybir.dt.bfloat16
    with tile.TileContext(nc) as tc:
        with (
            tc.tile_pool(name="xpool", bufs=n_ld + 2) as xpool,
            tc.tile_pool(name="wpool", bufs=6) as wpool,
        ):
            for rep, mb in ((r_, m_) for r_ in range(reps) for m_ in range(n_mb)):
                for i in range(n_ld):
                    xt = xpool.tile([P, KO_LD, m_block], bf16, tag="xt",
                                    name=f"x_{rep}_{mb}_{i}")
                    nc.sync.dma_start(xt, xb[mb, :, i * KO_LD:(i + 1) * KO_LD, :])
                for nc0 in range(n_nc):
                    c0 = nc0 * n_chunk
                    for ko in range(ko_n):
                        wt = wpool.tile([P, n_chunk], bf16, tag="wt")
                        nc.sync.dma_start(wt, wb[:, ko, c0:c0 + n_chunk])
    nc.compile()
    return nc


def _build_bf1_nodma(k, m_core, n, m_block=512, n_chunk=1024, reps=1):
    """Diagnostic: bf1's exact matmul stream with W loaded once (times pure PE)."""
    import concourse.mybir as mybir
    import concourse.tile as tile
    from concourse import bacc

    ko_n = k // P
    n_mb = m_core // m_block
    n_nc = n // n_chunk
    ms_n = m_block // P
    nt_n = n_chunk // MM_N
    n_ld = ko_n // KO_LD

    nc = bacc.Bacc(None, target_bir_lowering=False, debug=False)
    xb = nc.declare_dram_parameter("xb", [n_mb, P, ko_n, m_block],
                                   mybir.dt.bfloat16, isOutput=False)
    wb = nc.declare_dram_parameter("wb", [P, ko_n, n],
                                   mybir.dt.bfloat16, isOutput=False)
    out = nc.declare_dram_parameter("out", [m_core, n], mybir.dt.float32,
                                    isOutput=True)
    f32 = mybir.dt.float32
    bf16 = mybir.dt.bfloat16
    with tile.TileContext(nc) as tc:
        with (
            tc.tile_pool(name="xpool", bufs=2) as xpool,
            tc.tile_pool(name="wpool", bufs=1) as wpool,
            tc.tile_pool(name="opool", bufs=4) as opool,
            tc.tile_pool(name="pspool", bufs=8, space="PSUM") as pspool,
        ):
            xt = xpool.tile([P, KO_LD, m_block], bf16, tag="xt")
            nc.sync.dma_start(xt, xb[0, :, 0:KO_LD, :])
            wt = wpool.tile([P, n_chunk], bf16, tag="wt")
            nc.sync.dma_start(wt, wb[:, 0, 0:n_chunk])
            for rep, mb in ((r_, m_) for r_ in range(reps) for m_ in range(n_mb)):
                m0 = mb * m_block
                for nc0 in range(n_nc):
                    c0 = nc0 * n_chunk
                    psums = [
                        pspool.tile([P, MM_N], f32, tag="ps",
                                    name=f"ps_{rep}_{mb}_{nc0}_{i}")
                        for i in range(ms_n * nt_n)
                    ]
                    for ko in range(ko_n):
                        first = ko == 0
                        last = ko == ko_n - 1
                        kj = ko % KO_LD
                        for ms in range(ms_n):
                            lhs = xt[:, kj, ms * P:(ms + 1) * P]
                            for nt in range(nt_n):
                                nc.tensor.matmul(
                                    psums[ms * nt_n + nt],
                                    lhs,
                                    wt[:, nt * MM_N:(nt + 1) * MM_N],
                                    start=first,
                                    stop=last,
                                )
                    for ms in range(ms_n):
                        for nt in range(nt_n):
                            st = opool.tile([P, MM_N], f32, tag="st")
                            nc.vector.tensor_copy(out=st, in_=psums[ms * nt_n + nt])
                            nc.sync.dma_start(
                                out[m0 + ms * P:m0 + (ms + 1) * P,
                                    c0 + nt * MM_N:c0 + (nt + 1) * MM_N],
                                st,
                            )
    nc.compile()
    return nc


_BUILDERS = {
    "bf1": _build_bf1,
    "fp8": _build_fp8,
    "fp8b": lambda k, m, n, **kw: _build_fp8(k, m, n, m_block=512, n_chunk=1024,
                                             w_ld=8, **kw),
    "fp8c": lambda k, m, n, **kw: _build_fp8(k, m, n, m_block=512, n_chunk=1024,
                                             w_ld=16, **kw),
    "bf1_nomm": _build_bf1_nomm,
    "bf1_nodma": _build_bf1_nodma,
}

# variant -> (m_block for host x layout, operand dtype, W pre-scale)
VARIANT_CFG = {
    "bf1": (512, BF16, 1.0),
    "fp8": (256, E4M3, W_SCALE),
    "fp8b": (512, E4M3, W_SCALE),
    "fp8c": (512, E4M3, W_SCALE),
    "bf1_nomm": (512, BF16, 1.0),
    "bf1_nodma": (512, BF16, 1.0),
}


def _variant():
    return os.environ.get("KERNEL_VARIANT", "bf1")


def _get_nc(k, m_core, n, **kw):
    variant = _variant()
    key = (variant, k, m_core, n, tuple(sorted(kw.items())))
    if key not in _BUILD_CACHE:
        _BUILD_CACHE[key] = _BUILDERS[variant](k, m_core, n, **kw)
    return _BUILD_CACHE[key]


def _to_pkm_blocks(a, m_block, dtype):
    """[rows, k] fp32 -> contiguous [n_mb, P, ko_n, m_block] in `dtype`
    (k = ko*128 + p)."""
    rows, k = a.shape
    n_mb = rows // m_block
    ko_n = k // P
    a = a.astype(dtype)
    a = a.reshape(n_mb, m_block, ko_n, P).transpose(0, 3, 2, 1)
    return np.ascontiguousarray(a)


def _w_to_pkn(w, dtype, scale=1.0):
    """[n, k] fp32 -> contiguous [P, ko_n, n] in `dtype`."""
    n, k = w.shape
    ko_n = k // P
    if scale != 1.0:
        w = w * np.float32(scale)
    w = w.astype(dtype)
    w = w.reshape(n, ko_n, P).transpose(2, 1, 0)
    return np.ascontiguousarray(w)


def _make_runner(nc):
    """Build the sharded PJRT executor for `nc` across the 8 cores.

    Mirrors concourse.bass2jax.run_bass_via_pjrt, but returns a reusable
    closure so repeated calls share one jit cache and inputs can stay
    device-resident for benchmarking.
    """
    import jax
    import concourse.mybir as mybir
    from concourse import bass2jax
    from jax.experimental.shard_map import shard_map
    from jax.sharding import Mesh, NamedSharding, PartitionSpec

    bass2jax.install_neuronx_cc_hook()

    partition_name = nc.partition_id_tensor.name if nc.partition_id_tensor else None
    assert nc.dbg_addr is None

    in_names, out_names, out_avals = [], [], []
    for alloc in nc.m.functions[0].allocations:
        if not isinstance(alloc, mybir.MemoryLocationSet):
            continue
        name = alloc.memorylocations[0].name
        if alloc.kind == "ExternalInput":
            if name != partition_name:
                in_names.append(name)
        elif alloc.kind == "ExternalOutput":
            out_names.append(name)
            out_avals.append(
                jax.core.ShapedArray(tuple(alloc.tensor_shape), mybir.dt.np(alloc.dtype))
            )
    n_params = len(in_names)
    n_outs = len(out_avals)
    all_in_names = tuple(in_names) + tuple(out_names)
    if partition_name is not None:
        all_in_names = all_in_names + (partition_name,)
    donate = tuple(range(n_params, n_params + n_outs))

    def _body(*args):
        operands = list(args)
        if partition_name is not None:
            operands.append(bass2jax.partition_id_tensor())
        outs = bass2jax._bass_exec_p.bind(
            *operands,
            out_avals=tuple(out_avals),
            in_names=all_in_names,
            out_names=tuple(out_names),
            lowering_input_output_aliases=(),
            sim_require_finite=True,
            sim_require_nnan=True,
            nc=nc,
        )
        return tuple(outs)

    devices = jax.devices()[:N_CORES]
    assert len(devices) == N_CORES
    mesh = Mesh(np.asarray(devices), ("core",))
    spec = PartitionSpec("core")
    sharded = jax.jit(
        shard_map(
            _body,
            mesh=mesh,
            in_specs=(spec,) * (n_params + n_outs),
            out_specs=(spec,) * n_outs,
            check_rep=False,
        ),
        donate_argnums=donate,
        keep_unused=True,
    )
    sharding = NamedSharding(mesh, spec)
    return {
        "sharded": sharded,
        "sharding": sharding,
        "in_names": in_names,
        "out_names": out_names,
        "out_avals": out_avals,
        "n_params": n_params,
        "n_outs": n_outs,
    }


def _get_runner(nc):
    key = id(nc)
    if key not in _RUNNER_CACHE:
        _RUNNER_CACHE[key] = _make_runner(nc)
    return _RUNNER_CACHE[key]


def _run(nc, in_maps):
    """Execute the kernel across 8 cores; returns per-core output dicts."""
    import numpy as np

    r = _get_runner(nc)
    n_cores = len(in_maps)
    concat_in = [
        np.concatenate([np.asarray(m[name]) for m in in_maps], axis=0)
        for name in r["in_names"]
    ]
    concat_zeros = [
        np.zeros((n_cores * a.shape[0], *a.shape[1:]), a.dtype) for a in r["out_avals"]
    ]
    out_arrs = r["sharded"](*concat_in, *concat_zeros)
    return [
        {
            name: np.asarray(out_arrs[i]).reshape(n_cores, *r["out_avals"][i].shape)[c]
            for i, name in enumerate(r["out_names"])
        }
        for c in range(n_cores)
    ]


def _bench(in_maps, k, m_core, n, reps):
    """Measure steady-state per-GEMM time: the kernel repeated `reps` times
    inside one program, minus the reps=1 program, divided by reps-1. Fixed
    dispatch overhead cancels in the difference. Sets LAST_EXEC_NS."""
    global LAST_EXEC_NS
    import time

    import jax
    import jax.numpy as jnp
    import numpy as np

    times = {}
    dev_in = None
    for r_reps in (1, reps):
        nc = _get_nc(k, m_core, n, reps=r_reps)
        r = _get_runner(nc)
        if dev_in is None:
            concat_in = [
                np.concatenate([np.asarray(m[name]) for m in in_maps], axis=0)
                for name in r["in_names"]
            ]
            dev_in = [jax.device_put(a, r["sharding"]) for a in concat_in]
            jax.block_until_ready(dev_in)

        def _zeros():
            zs = [
                jax.jit(lambda a=a: jnp.zeros(a.shape, a.dtype),
                        out_shardings=r["sharding"])()
                for a in r["out_avals"]
            ]
            jax.block_until_ready(zs)
            return zs

        out = r["sharded"](*dev_in, *_zeros())  # compile + warmup
        jax.block_until_ready(out)
        attempts = []
        for _ in range(int(os.environ.get("KERNEL_BENCH_TRIES", "5"))):
            zs = _zeros()
            t0 = time.perf_counter()
            out = r["sharded"](*dev_in, *zs)
            jax.block_until_ready(out)
            attempts.append(time.perf_counter() - t0)
        times[r_reps] = min(attempts)
        print(f"[bench] reps={r_reps}: best {min(attempts) * 1e3:.3f} ms  "
              f"all {[f'{a * 1e3:.2f}' for a in attempts]}")

    per_iter = (times[reps] - times[1]) / (reps - 1)
    LAST_EXEC_NS = int(per_iter * 1e9)
    print(f"[bench] per-GEMM: {per_iter * 1e3:.3f} ms "
          f"(fixed+1iter: {times[1] * 1e3:.3f} ms)")


def kernel(input_, weight, bias):
    global LAST_RESULTS

    input_ = np.asarray(input_, dtype=np.float32)
    weight = np.asarray(weight, dtype=np.float32)
    bias = np.asarray(bias, dtype=np.float32)

    seq, batch, k = input_.shape
    n = weight.shape[0]
    m_full = seq * batch
    m_core = m_full // N_CORES

    variant = _variant()
    nc = _get_nc(k, m_core, n)

    x2 = input_.reshape(m_full, k)
    m_block, dtype, w_scale = VARIANT_CFG[variant]
    wp = _w_to_pkn(weight, dtype, scale=w_scale)

    in_maps = []
    for c in range(N_CORES):
        xp = _to_pkm_blocks(x2[c * m_core:(c + 1) * m_core], m_block, dtype)
        in_maps.append({"xb": xp, "wb": wp})

    results = _run(nc, in_maps)
    LAST_RESULTS = results

    bench_reps = int(os.environ.get("KERNEL_BENCH", "0"))
    if bench_reps > 1:
        _bench(in_maps, k, m_core, n, bench_reps)

    out = np.concatenate([results[c]["out"] for c in range(N_CORES)], axis=0)
    if w_scale != 1.0:
        out = out * np.float32(1.0 / w_scale)
    out = out.reshape(seq, batch, n)
    if bias.any():
        out = out + bias
    return out
